# revision 1
# baseline (speedup 1.0000x reference)
"""GCN message-passing Bass kernel for TRN2 (8 cores).

Math: delta = segment_sum(w_e * x[src_e]) @ W^T   (linearity: transform after aggregate)

Sharding: targets split across 8 cores (12500 each). Per core, targets are
degree-sorted and grouped into 128-target blocks; each block-j target p has
D_j padded edge slots. One indirect DMA gathers x rows for a whole block:
out[p, d, :] = x[idx[p, d], :]  (pad slots point at row 0, weight 0).
DVE multiplies by per-slot weights (broadcast AP) and reduces over slots.
PE transposes agg and applies W^T; indirect DMA scatters final rows.
"""

import math
from contextlib import ExitStack

import numpy as np

import concourse.bass as bass
import concourse.bacc as bacc
import concourse.mybir as mybir
import concourse.tile as tile
from concourse.bass import IndirectOffsetOnAxis
from concourse.bass_utils import run_bass_kernel_spmd

P = 128
N_CORES = 8
F32 = mybir.dt.float32
I32 = mybir.dt.int32


def preprocess(source, target, edge_weights, n_nodes, n_cores=N_CORES):
    """Build per-core gather/weight/target-id arrays and the shared block schedule.

    Returns dict with:
      d_sched: list[int] per-block slot count (same for all cores)
      per_core: list of dicts with idx_all [128,S] i32, w_all [128,S] f32,
                tgt_all [128,nblk] i32
      nt: targets per core, nblk: blocks per core
    """
    source = np.asarray(source).astype(np.int64)
    target = np.asarray(target).astype(np.int64)
    edge_weights = np.asarray(edge_weights).astype(np.float32)
    nt = n_nodes // n_cores
    assert nt * n_cores == n_nodes
    nblk = math.ceil(nt / P)

    cores = []
    for k in range(n_cores):
        lo, hi = k * nt, (k + 1) * nt
        m = (target >= lo) & (target < hi)
        src_k = source[m]
        w_k = edge_weights[m]
        tl_k = target[m] - lo  # local target ids

        deg = np.bincount(tl_k, minlength=nt)
        perm = np.argsort(deg, kind="stable")  # local ids, degree-ascending
        # block j holds targets perm[j*128:(j+1)*128]; slot count = max degree in block
        deg_sorted = deg[perm]
        d_k = [int(deg_sorted[j * P : (j + 1) * P].max()) if j * P < nt else 0
               for j in range(nblk)]
        cores.append(dict(src=src_k, w=w_k, tl=tl_k, deg=deg, perm=perm, d_k=d_k))

    d_sched = [max(c["d_k"][j] for c in cores) for j in range(nblk)]
    S = sum(d_sched)
    offs = np.concatenate([[0], np.cumsum(d_sched)]).astype(np.int64)

    per_core = []
    for k in range(n_cores):
        c = cores[k]
        # CSR by local target id
        order = np.argsort(c["tl"], kind="stable")
        src_s, w_s = c["src"][order], c["w"][order]
        starts = np.concatenate([[0], np.cumsum(c["deg"])]).astype(np.int64)

        # Paired gather: idx addresses row-pairs of x viewed as [n/2, 128];
        # each slot expands to two weight columns (even/odd row of the pair).
        idx_all = np.zeros((P, S), dtype=np.int32)  # pad -> pair 0 (weights 0)
        w_all = np.zeros((P, 2 * S), dtype=np.float32)
        tgt_all = np.full((P, nblk), 1 << 20, dtype=np.int32)  # pad -> OOB skip
        perm = c["perm"]
        for j in range(nblk):
            o = offs[j]
            blk = perm[j * P : (j + 1) * P]
            for p, t in enumerate(blk):
                s0, d = starts[t], c["deg"][t]
                if d:
                    sl = src_s[s0 : s0 + d]
                    idx_all[p, o : o + d] = sl >> 1
                    w_all[p, 2 * o + 2 * np.arange(d) + (sl & 1)] = w_s[s0 : s0 + d]
                tgt_all[p, j] = t
        per_core.append(dict(idx_all=idx_all, w_all=w_all, tgt_all=tgt_all))

    return dict(d_sched=d_sched, S=S, per_core=per_core, nt=nt, nblk=nblk)


def build_nc(d_sched, S, n_nodes, nt, nblk, d_feat=64, bufs=3):
    nc = bacc.Bacc("TRN2", target_bir_lowering=False, debug=False)
    D = d_feat
    x_t = nc.dram_tensor("x", [n_nodes // 2, 2 * D], F32, kind="ExternalInput")
    wt_t = nc.dram_tensor("wT", [D, D], F32, kind="ExternalInput")
    idx_t = nc.dram_tensor("idx", [P, S], I32, kind="ExternalInput")
    wgt_t = nc.dram_tensor("wgt", [P, 2 * S], F32, kind="ExternalInput")
    tgt_t = nc.dram_tensor("tgt", [P, nblk], I32, kind="ExternalInput")
    eye_t = nc.dram_tensor("eye", [P, P], F32, kind="ExternalInput")
    out_t = nc.dram_tensor("out", [nt, D], F32, kind="ExternalOutput")

    with tile.TileContext(nc) as tc, ExitStack() as ctx:
        const = ctx.enter_context(tc.tile_pool(name="const", bufs=1))
        gpool = ctx.enter_context(tc.tile_pool(name="gather", bufs=bufs))
        mpool = ctx.enter_context(tc.tile_pool(name="msg", bufs=bufs))
        apool = ctx.enter_context(tc.tile_pool(name="agg", bufs=bufs))
        tpool = ctx.enter_context(tc.tile_pool(name="aggT", bufs=bufs))
        dpool = ctx.enter_context(tc.tile_pool(name="delta", bufs=bufs))
        psum = ctx.enter_context(tc.tile_pool(name="psum", bufs=4, space="PSUM"))

        ident = const.tile([P, P], F32)
        nc.sync.dma_start(out=ident[:], in_=eye_t.ap())
        wt_sb = const.tile([D, D], F32)
        nc.sync.dma_start(out=wt_sb[:], in_=wt_t.ap())
        idx_sb = const.tile([P, S], I32)
        nc.sync.dma_start(out=idx_sb[:], in_=idx_t.ap())
        wgt_sb = const.tile([P, 2 * S], F32)
        nc.sync.dma_start(out=wgt_sb[:], in_=wgt_t.ap())
        tgt_sb = const.tile([P, nblk], I32)
        nc.sync.dma_start(out=tgt_sb[:], in_=tgt_t.ap())
        bounds_reg = nc.gpsimd.to_reg(nt - 1)

        # Prime engines on the upfront loads so per-block instructions carry
        # at most one sync wait each (SEQ instruction structs encode one).
        prime = const.tile([P, 1], F32)
        nc.vector.tensor_copy(out=prime[:], in_=wgt_sb[:, :1])
        prime_ps = psum.tile([P, P], F32, tag="tp")
        nc.tensor.transpose(out=prime_ps[:], in_=ident[:], identity=ident[:])
        nc.tensor.transpose(out=prime_ps[:D, :D], in_=wt_sb[:], identity=ident[:D, :D])

        off = 0
        for j in range(nblk):
            dj = d_sched[j]
            agg = apool.tile([P, D], F32, tag="agg")
            if dj > 0:
                g = gpool.tile([P, dj * 2 * D], F32, tag="g")
                for dd in range(dj):
                    nc.gpsimd.indirect_dma_start(
                        out=g[:, dd * 2 * D : (dd + 1) * 2 * D],
                        out_offset=None,
                        in_=x_t.ap(),
                        in_offset=IndirectOffsetOnAxis(
                            ap=idx_sb[:, off + dd : off + dd + 1], axis=0
                        ),
                    )
                msg = mpool.tile([P, dj * 2 * D], F32, tag="m")
                nc.vector.tensor_tensor(
                    out=msg[:].rearrange("p (d o) -> p d o", o=D),
                    in0=g[:].rearrange("p (d o) -> p d o", o=D),
                    in1=wgt_sb[:, 2 * off : 2 * (off + dj)].to_broadcast(
                        [P, 2 * dj, D]
                    ),
                    op=mybir.AluOpType.mult,
                )
                nc.vector.tensor_reduce(
                    out=agg[:],
                    in_=msg[:].rearrange("p (d o) -> p o d", o=D),
                    axis=mybir.AxisListType.X,
                    op=mybir.AluOpType.add,
                )
            else:
                nc.vector.memset(agg[:], 0.0)

            agg_ps = psum.tile([D, P], F32, tag="tp")
            nc.tensor.transpose(out=agg_ps[:], in_=agg[:], identity=ident[:])
            agg_tr = tpool.tile([D, P], F32, tag="aT")
            nc.vector.tensor_copy(out=agg_tr[:], in_=agg_ps[:])

            d_ps = psum.tile([P, D], F32, tag="mm")
            nc.tensor.matmul(out=d_ps[:], lhsT=agg_tr[:], rhs=wt_sb[:], start=True, stop=True)
            d_sb = dpool.tile([P, D], F32, tag="d")
            nc.vector.tensor_copy(out=d_sb[:], in_=d_ps[:])

            nc.gpsimd.indirect_dma_start(
                out=out_t.ap(),
                out_offset=IndirectOffsetOnAxis(ap=tgt_sb[:, j : j + 1], axis=0),
                in_=d_sb[:],
                in_offset=None,
                bounds_check=bounds_reg,
                oob_is_err=False,
            )
            off += dj
    nc.compile()
    return nc


def run_gcn(x, W, edge_weights, source, target, num_nodes, trace=False, bufs=3):
    """Full-input host entry: preprocess, build, run on 8 cores, assemble output."""
    n_nodes = int(num_nodes)
    pp = preprocess(source, target, edge_weights, n_nodes)
    nc = build_nc(pp["d_sched"], pp["S"], n_nodes, pp["nt"], pp["nblk"],
                  d_feat=x.shape[1], bufs=bufs)
    x_np = np.ascontiguousarray(np.asarray(x), dtype=np.float32).reshape(
        n_nodes // 2, 2 * x.shape[1]
    )
    wt_np = np.ascontiguousarray(np.asarray(W).T, dtype=np.float32)
    in_maps = []
    for k in range(N_CORES):
        pc = pp["per_core"][k]
        in_maps.append({
            "x": x_np, "wT": wt_np, "eye": np.eye(P, dtype=np.float32),
            "idx": pc["idx_all"], "wgt": pc["w_all"], "tgt": pc["tgt_all"],
        })
    res = run_bass_kernel_spmd(nc, in_maps, core_ids=list(range(N_CORES)), trace=trace)
    out = np.concatenate([res.results[k]["out"] for k in range(N_CORES)], axis=0)
    return out, res


def kernel(**inputs) -> np.ndarray:
    """Harness entry: full unsharded inputs -> full (num_nodes, 64) output."""
    out, _ = run_gcn(
        np.asarray(inputs["x"]),
        np.asarray(inputs["W"]),
        np.asarray(inputs["edge_weights"]),
        np.asarray(inputs["source"]),
        np.asarray(inputs["target"]),
        int(inputs["num_nodes"]),
        trace=False,
    )
    return out



# revision 5
# speedup vs baseline: 18.8479x; 18.8479x over previous
"""GCN message-passing Bass kernel for TRN2 (8 cores).

Math: delta = segment_sum(w_e * x[src_e]) @ W^T   (linearity: transform after aggregate)

Sharding: targets split across 8 cores (12500 each). Per core, targets are
degree-sorted and grouped into 128-target blocks; block j gives each of its
128 targets D_j padded edge slots (pad -> weight 0). Blocks are packed into
gathers of <= 64 slots (8192 indices).

Per gather: ONE InstDMAGatherAnt pulls bf16 x rows from a per-gather DRAM
table (the unique x rows of that gather's slot entries, host-reindexed to
int16 ids, rows padded to 256B stride with a 128B payload):
  gt[p, s*64:(s+1)*64] = table[idx[s*128+p], :64]
DVE multiplies the gather in-place by per-slot weights (broadcast AP).
PE does the slot reduction: per block, dj accumulating matmuls
(lhsT=slot tile, rhs=identity) sum slot^T into PSUM [64,128]; Act copies the
f32 PSUM to bf16 SBUF; PE applies W^T (aggT as lhsT); Act copies the result
into a per-gather output tile written to DRAM contiguously in block order
(plain DMA, no scatter). The host applies the inverse target permutation to
assemble the final (num_nodes, 64) output.
"""

import math
from contextlib import ExitStack

import numpy as np
import ml_dtypes

import concourse.bass as bass
import concourse.bacc as bacc
import concourse.mybir as mybir
import concourse.tile as tile
from concourse.bass_utils import run_bass_kernel_spmd
from concourse.library_config import mlp as mlp_library

P = 128
N_CORES = 8
D = 64
F32 = mybir.dt.float32
BF16 = mybir.dt.bfloat16
I16 = mybir.dt.int16
NP_BF16 = np.dtype(ml_dtypes.bfloat16)

GATHER_SLOTS = 64  # max slots per dma_gather (64*128 = 8192 indices)
TABLE_ROWS = 8192  # per-gather unique-row table size (int16-indexable)


def preprocess(source, target, edge_weights, n_nodes, n_cores=N_CORES):
    """Build per-core gather tables/indices/weights and the shared schedule.

    Returns dict with:
      d_sched: [nblk] per-block slot count (same for all cores)
      gathers: list of (blo, bhi, s0, gsz) gather groups over blocks
      per_core: dict with tables [G,TABLE_ROWS,128] bf16, idx16 [128,S*8] i16,
                w_all [128,S] bf16, perm_pad [nblk*128] i64
      nt, nblk, S
    """
    source = np.asarray(source).astype(np.int64)
    target = np.asarray(target).astype(np.int64)
    edge_weights = np.asarray(edge_weights).astype(np.float32)
    nt = n_nodes // n_cores
    assert nt * n_cores == n_nodes
    nblk = math.ceil(nt / P)
    ntp = nblk * P

    cores = []
    d_sched = np.zeros(nblk, dtype=np.int64)
    for k in range(n_cores):
        lo, hi = k * nt, (k + 1) * nt
        m = (target >= lo) & (target < hi)
        src_k = source[m]
        w_k = edge_weights[m]
        tl_k = target[m] - lo  # local target ids

        deg = np.bincount(tl_k, minlength=nt)
        perm = np.argsort(deg, kind="stable")  # local ids, degree-ascending
        deg_pad = np.concatenate([deg[perm], np.zeros(ntp - nt, dtype=deg.dtype)])
        d_k = deg_pad.reshape(nblk, P).max(axis=1)
        d_sched = np.maximum(d_sched, d_k)
        cores.append(dict(src=src_k, w=w_k, tl=tl_k, deg=deg, perm=perm))

    offs = np.concatenate([[0], np.cumsum(d_sched)]).astype(np.int64)
    S = int(offs[-1])

    # pack blocks into gathers of <= GATHER_SLOTS slots
    gathers = []
    blo = 0
    while blo < nblk:
        bhi = blo
        gsz = 0
        while bhi < nblk and (bhi == blo or gsz + d_sched[bhi] <= GATHER_SLOTS):
            gsz += int(d_sched[bhi])
            bhi += 1
        gathers.append((blo, bhi, int(offs[blo]), gsz))
        blo = bhi
    G = len(gathers)

    per_core = []
    for k in range(n_cores):
        c = cores[k]
        deg, perm = c["deg"], c["perm"]
        rank = np.empty(nt, dtype=np.int64)
        rank[perm] = np.arange(nt)

        order = np.argsort(c["tl"], kind="stable")
        tls = c["tl"][order]
        srcs = c["src"][order]
        ws = c["w"][order]
        starts = np.cumsum(deg) - deg  # first edge position per target
        eo = np.arange(len(tls)) - starts[tls]  # occurrence index within target
        rr = rank[tls]
        pp = rr & (P - 1)
        bb = rr >> 7
        col = offs[bb] + eo

        entries = np.zeros((P, S), dtype=np.int64)  # pad -> x row 0 (weight 0)
        w_all = np.zeros((P, S), dtype=NP_BF16)
        entries[pp, col] = srcs
        w_all[pp, col] = ws.astype(NP_BF16)

        # per-gather unique tables + int16 indices, wrapped for the Q7 layout
        uniq_list = []
        idx16 = np.empty((P, S * 8), dtype=np.int16)
        for gi, (_, _, s0, gsz) in enumerate(gathers):
            ent = entries[:, s0 : s0 + gsz]
            uniq, inv = np.unique(ent, return_inverse=True)
            assert len(uniq) <= TABLE_ROWS
            uniq_list.append(uniq)
            inv = inv.reshape(P, gsz).astype(np.int16)
            iflat = inv.T.reshape(-1)  # position i = s_local*128 + p
            blkcols = np.tile(iflat.reshape(gsz * 8, 16).T, (8, 1))
            idx16[:, s0 * 8 : (s0 + gsz) * 8] = blkcols

        perm_pad = np.full(ntp, -1, dtype=np.int64)
        perm_pad[:nt] = perm
        per_core.append(dict(uniq_list=uniq_list, idx16=idx16, w_all=w_all,
                             perm_pad=perm_pad))

    return dict(d_sched=[int(d) for d in d_sched], S=S, gathers=gathers,
                per_core=per_core, nt=nt, nblk=nblk, G=G)


def _dma_gather(gp, out_ap, in_ap, idxs_ap, num_idxs):
    """InstDMAGatherAnt with a 128B payload at 256B row stride (elem_size=64
    bf16, stride_bytes_256=1). bass.dma_gather asserts elem%256B, but the Q7
    ucode handles 128B payloads (verified on HW); construct directly."""
    _in_ap = gp.lower_ap_dma(in_ap, for_custom_bir_dma=True)
    _idxs_ap = gp.lower_ap(idxs_ap)
    _out_ap = gp.lower_ap(out_ap)
    return gp.add_instruction(
        mybir.InstDMAGatherAnt(
            name=gp.bass.get_next_instruction_name(),
            ins=[*_in_ap, _idxs_ap, gp.lower_val_access(gp.to_reg(num_idxs))],
            outs=[_out_ap],
            transpose=False,
            num_idxs=num_idxs,
            elem_size=D,
            stride_bytes_256=1,
            gen_mode=0,
            single_packet=False,
            queue_num=0,
            sbuf_tokens_per_rank=0,
            sbuf_free_dim_per_rank=0,
            sbuf_free_dim_pad_per_rank=0,
            sbuf_byte_offset=0,
        )
    )


def build_nc(pp, n_nodes, bufs=3):
    d_sched, S, nblk, gathers = pp["d_sched"], pp["S"], pp["nblk"], pp["gathers"]
    nc = bacc.Bacc("TRN2", target_bir_lowering=False, debug=False)
    tabs = [nc.dram_tensor(f"xg{gi}", [TABLE_ROWS, 2 * D], BF16, kind="ExternalInput")
            for gi in range(len(gathers))]
    wt_t = nc.dram_tensor("wT", [D, D], BF16, kind="ExternalInput")
    idx_t = nc.dram_tensor("idx", [P, S * 8], I16, kind="ExternalInput")
    wgt_t = nc.dram_tensor("wgt", [P, S], BF16, kind="ExternalInput")
    eye_t = nc.dram_tensor("eye", [P, P], BF16, kind="ExternalInput")
    out_t = nc.dram_tensor("out", [P, nblk * D], F32, kind="ExternalOutput")

    with tile.TileContext(nc) as tc, ExitStack() as ctx:
        nc.gpsimd.load_library(mlp_library)
        const = ctx.enter_context(tc.tile_pool(name="const", bufs=1))
        gpool = ctx.enter_context(tc.tile_pool(name="gather", bufs=bufs))
        tpool = ctx.enter_context(tc.tile_pool(name="aggT", bufs=4))
        dpool = ctx.enter_context(tc.tile_pool(name="delta", bufs=bufs))
        psA = ctx.enter_context(tc.tile_pool(name="psA", bufs=4, space="PSUM"))
        psB = ctx.enter_context(tc.tile_pool(name="psB", bufs=4, space="PSUM"))

        ident = const.tile([P, P], BF16)
        nc.sync.dma_start(out=ident[:], in_=eye_t.ap())
        wt_sb = const.tile([D, D], BF16)
        nc.sync.dma_start(out=wt_sb[:], in_=wt_t.ap())
        idx_sb = const.tile([P, S * 8], I16)
        nc.sync.dma_start(out=idx_sb[:], in_=idx_t.ap())
        wgt_sb = const.tile([P, S], BF16)
        nc.sync.dma_start(out=wgt_sb[:], in_=wgt_t.ap())

        # Prime engines on the upfront loads so per-block instructions carry
        # at most one sync wait each (SEQ instruction structs encode one).
        prime = const.tile([P, 1], BF16)
        nc.vector.tensor_copy(out=prime[:], in_=wgt_sb[:, :1])
        prime2 = const.tile([P, 1], BF16)
        nc.scalar.copy(out=prime2[:], in_=ident[:, :1])
        prime_ps = psA.tile([D, P], F32, tag="agg")
        nc.tensor.matmul(out=prime_ps[:], lhsT=ident[:, :D], rhs=ident[:],
                         start=True, stop=True)

        for gi, (blo, bhi, s0, gsz) in enumerate(gathers):
            gt = gpool.tile([P, gsz * D], BF16, tag="g")
            _dma_gather(
                nc.gpsimd,
                gt[:].rearrange("p (c e) -> p c e", e=D),
                tabs[gi].ap(),
                idx_sb[:, s0 * 8 : (s0 + gsz) * 8],
                gsz * P,
            )
            nc.vector.tensor_tensor(
                out=gt[:].rearrange("p (d o) -> p d o", o=D),
                in0=gt[:].rearrange("p (d o) -> p d o", o=D),
                in1=wgt_sb[:, s0 : s0 + gsz].to_broadcast([P, gsz, D]),
                op=mybir.AluOpType.mult,
            )

            ng = bhi - blo
            dgrp = dpool.tile([P, ng * D], F32, tag="d")
            lo = 0
            for j in range(blo, bhi):
                dj = d_sched[j]
                agg_ps = psA.tile([D, P], F32, tag="agg")
                for dd in range(dj):
                    nc.tensor.matmul(
                        out=agg_ps[:],
                        lhsT=gt[:, (lo + dd) * D : (lo + dd + 1) * D],
                        rhs=ident[:],
                        start=(dd == 0),
                        stop=(dd == dj - 1),
                    )
                aggT = tpool.tile([D, P], BF16, tag="aT")
                nc.scalar.copy(out=aggT[:], in_=agg_ps[:])
                d_ps = psB.tile([P, D], F32, tag="mm")
                nc.tensor.matmul(out=d_ps[:], lhsT=aggT[:], rhs=wt_sb[:],
                                 start=True, stop=True)
                nc.scalar.copy(out=dgrp[:, (j - blo) * D : (j - blo + 1) * D],
                               in_=d_ps[:])
                lo += dj

            nc.sync.dma_start(out=out_t.ap()[:, blo * D : bhi * D], in_=dgrp[:])
    nc.compile()
    return nc


def run_gcn(x, W, edge_weights, source, target, num_nodes, trace=False, bufs=3):
    """Full-input host entry: preprocess, build, run on 8 cores, assemble output."""
    n_nodes = int(num_nodes)
    pp = preprocess(source, target, edge_weights, n_nodes)
    nc = build_nc(pp, n_nodes, bufs=bufs)
    x_bf = np.ascontiguousarray(np.asarray(x, dtype=np.float32)).astype(NP_BF16)
    wt_np = np.ascontiguousarray(np.asarray(W, dtype=np.float32).T).astype(NP_BF16)
    eye_np = np.eye(P, dtype=np.float32).astype(NP_BF16)
    in_maps = []
    for k in range(N_CORES):
        pc = pp["per_core"][k]
        im = {"wT": wt_np, "eye": eye_np, "idx": pc["idx16"], "wgt": pc["w_all"]}
        for gi, uniq in enumerate(pc["uniq_list"]):
            tab = np.zeros((TABLE_ROWS, 2 * D), dtype=NP_BF16)
            tab[: len(uniq), :D] = x_bf[uniq]
            im[f"xg{gi}"] = tab
        in_maps.append(im)
    res = run_bass_kernel_spmd(nc, in_maps, core_ids=list(range(N_CORES)), trace=trace)

    nt, nblk = pp["nt"], pp["nblk"]
    out = np.empty((n_nodes, D), dtype=np.float32)
    for k in range(N_CORES):
        raw = np.asarray(res.results[k]["out"], dtype=np.float32)  # [128, nblk*64]
        cube = raw.reshape(P, nblk, D).transpose(1, 0, 2).reshape(nblk * P, D)
        perm_pad = pp["per_core"][k]["perm_pad"]
        valid = perm_pad >= 0
        out[k * nt + perm_pad[valid]] = cube[valid]
    return out, res


def kernel(**inputs) -> np.ndarray:
    """Harness entry: full unsharded inputs -> full (num_nodes, 64) output."""
    out, _ = run_gcn(
        np.asarray(inputs["x"]),
        np.asarray(inputs["W"]),
        np.asarray(inputs["edge_weights"]),
        np.asarray(inputs["source"]),
        np.asarray(inputs["target"]),
        int(inputs["num_nodes"]),
        trace=False,
    )
    return out


# revision 23
# speedup vs baseline: 19.8796x; 1.0547x over previous
"""GCN message-passing Bass kernel for TRN2 (8 cores).

Math: delta = segment_sum(w_e * x[src_e]) @ W^T   (linearity: transform after aggregate)

Sharding: targets split across 8 cores (12500 each). Per core, targets are
degree-sorted and grouped into 128-target blocks; block j gives each of its
128 targets D_j padded edge slots (pad -> weight 0). Blocks are packed into
gathers of <= 64 slots (8192 indices).

Per gather: ONE InstDMAGatherAnt pulls bf16 x rows from a per-gather DRAM
table (the unique x rows of that gather's slot entries, host-reindexed to
int16 ids, rows padded to 256B stride with a 128B payload):
  gt[p, s*64:(s+1)*64] = table[idx[s*128+p], :64]
DVE multiplies the gather in-place by per-slot weights (broadcast AP).
PE does the slot reduction: per block, dj accumulating matmuls
(lhsT=slot tile, rhs=identity) sum slot^T into PSUM [64,128]; Act copies the
f32 PSUM to bf16 SBUF; PE applies W^T (aggT as lhsT); Act copies the result
into a per-gather output tile written to DRAM contiguously in block order
(plain DMA, no scatter). The host applies the inverse target permutation to
assemble the final (num_nodes, 64) output.
"""

import math
from contextlib import ExitStack

import numpy as np
import ml_dtypes

import concourse.bass as bass
import concourse.bacc as bacc
import concourse.mybir as mybir
import concourse.tile as tile
from concourse.bass_utils import run_bass_kernel_spmd
from concourse.library_config import mlp as mlp_library

P = 128
N_CORES = 8
D = 64
F32 = mybir.dt.float32
BF16 = mybir.dt.bfloat16
I16 = mybir.dt.int16
NP_BF16 = np.dtype(ml_dtypes.bfloat16)

GATHER_SLOTS = 64  # max slots per dma_gather (64*128 = 8192 indices)
TABLE_ROWS = 8192  # per-gather unique-row table size (int16-indexable)


def preprocess(source, target, edge_weights, n_nodes, n_cores=N_CORES):
    """Build per-core gather tables/indices/weights and the shared schedule.

    Returns dict with:
      d_sched: [nblk] per-block slot count (same for all cores)
      gathers: list of (blo, bhi, s0, gsz) gather groups over blocks
      per_core: dict with tables [G,TABLE_ROWS,128] bf16, idx16 [128,S*8] i16,
                w_all [128,S] bf16, perm_pad [nblk*128] i64
      nt, nblk, S
    """
    source = np.asarray(source).astype(np.int64)
    target = np.asarray(target).astype(np.int64)
    edge_weights = np.asarray(edge_weights).astype(np.float32)
    nt = n_nodes // n_cores
    assert nt * n_cores == n_nodes
    nblk = math.ceil(nt / P)
    ntp = nblk * P

    cores = []
    d_sched = np.zeros(nblk, dtype=np.int64)
    for k in range(n_cores):
        lo, hi = k * nt, (k + 1) * nt
        m = (target >= lo) & (target < hi)
        src_k = source[m]
        w_k = edge_weights[m]
        tl_k = target[m] - lo  # local target ids

        deg = np.bincount(tl_k, minlength=nt)
        perm = np.argsort(deg, kind="stable")  # local ids, degree-ascending
        deg_pad = np.concatenate([deg[perm], np.zeros(ntp - nt, dtype=deg.dtype)])
        d_k = deg_pad.reshape(nblk, P).max(axis=1)
        d_sched = np.maximum(d_sched, d_k)
        cores.append(dict(src=src_k, w=w_k, tl=tl_k, deg=deg, perm=perm))

    offs = np.concatenate([[0], np.cumsum(d_sched)]).astype(np.int64)
    S = int(offs[-1])

    # pack blocks into gathers of <= GATHER_SLOTS slots; keep the last few
    # gathers single-block so the un-overlapped pipeline tail stays short
    single_tail = 4
    gathers = []
    blo = 0
    while blo < nblk:
        bhi = blo
        gsz = 0
        while (bhi < nblk and (bhi == blo or gsz + d_sched[bhi] <= GATHER_SLOTS)
               and not (bhi > blo and bhi >= nblk - single_tail)):
            gsz += int(d_sched[bhi])
            bhi += 1
        gathers.append((blo, bhi, int(offs[blo]), gsz))
        blo = bhi
    G = len(gathers)

    per_core = []
    for k in range(n_cores):
        c = cores[k]
        deg, perm = c["deg"], c["perm"]
        rank = np.empty(nt, dtype=np.int64)
        rank[perm] = np.arange(nt)

        order = np.argsort(c["tl"], kind="stable")
        tls = c["tl"][order]
        srcs = c["src"][order]
        ws = c["w"][order]
        starts = np.cumsum(deg) - deg  # first edge position per target
        eo = np.arange(len(tls)) - starts[tls]  # occurrence index within target
        rr = rank[tls]
        pp = rr & (P - 1)
        bb = rr >> 7
        col = offs[bb] + eo

        entries = np.zeros((P, S), dtype=np.int64)  # pad -> x row 0 (weight 0)
        w_all = np.zeros((P, S), dtype=NP_BF16)
        entries[pp, col] = srcs
        w_all[pp, col] = ws.astype(NP_BF16)

        # per-gather unique tables + int16 indices, wrapped for the Q7 layout
        uniq_list = []
        idx16 = np.empty((P, S * 8), dtype=np.int16)
        for gi, (_, _, s0, gsz) in enumerate(gathers):
            ent = entries[:, s0 : s0 + gsz]
            uniq, inv = np.unique(ent, return_inverse=True)
            assert len(uniq) <= TABLE_ROWS
            uniq_list.append(uniq)
            inv = inv.reshape(P, gsz).astype(np.int16)
            iflat = inv.T.reshape(-1)  # position i = s_local*128 + p
            blkcols = np.tile(iflat.reshape(gsz * 8, 16).T, (8, 1))
            idx16[:, s0 * 8 : (s0 + gsz) * 8] = blkcols

        perm_pad = np.full(ntp, -1, dtype=np.int64)
        perm_pad[:nt] = perm
        per_core.append(dict(uniq_list=uniq_list, idx16=idx16, w_all=w_all,
                             perm_pad=perm_pad))

    return dict(d_sched=[int(d) for d in d_sched], S=S, gathers=gathers,
                per_core=per_core, nt=nt, nblk=nblk, G=G)


def _dma_gather(gp, out_ap, in_ap, idxs_ap, num_idxs):
    """InstDMAGatherAnt with a 128B payload at 256B row stride (elem_size=64
    bf16, stride_bytes_256=1). bass.dma_gather asserts elem%256B, but the Q7
    ucode handles 128B payloads (verified on HW); construct directly."""
    _in_ap = gp.lower_ap_dma(in_ap, for_custom_bir_dma=True)
    _idxs_ap = gp.lower_ap(idxs_ap)
    _out_ap = gp.lower_ap(out_ap)
    return gp.add_instruction(
        mybir.InstDMAGatherAnt(
            name=gp.bass.get_next_instruction_name(),
            ins=[*_in_ap, _idxs_ap, gp.lower_val_access(gp.to_reg(num_idxs))],
            outs=[_out_ap],
            transpose=False,
            num_idxs=num_idxs,
            elem_size=D,
            stride_bytes_256=1,
            gen_mode=0,
            single_packet=False,
            queue_num=0,
            sbuf_tokens_per_rank=0,
            sbuf_free_dim_per_rank=0,
            sbuf_free_dim_pad_per_rank=0,
            sbuf_byte_offset=0,
        )
    )


def build_nc(pp, n_nodes, bufs=4, out_bf16=True, psum_bufs=4, stages=3,
             aggT_on_dve=False):
    # stages: 1=gather only, 2=+mult, 3=full (ablation knob for timing)
    d_sched, S, nblk, gathers = pp["d_sched"], pp["S"], pp["nblk"], pp["gathers"]
    nc = bacc.Bacc("TRN2", target_bir_lowering=False, debug=False)
    tabs = [nc.dram_tensor(f"xg{gi}", [TABLE_ROWS, 2 * D], BF16, kind="ExternalInput")
            for gi in range(len(gathers))]
    wt_t = nc.dram_tensor("wT", [D, D], BF16, kind="ExternalInput")
    idx_t = nc.dram_tensor("idx", [P, S * 8], I16, kind="ExternalInput")
    wgt_t = nc.dram_tensor("wgt", [P, S], BF16, kind="ExternalInput")
    eye_t = nc.dram_tensor("eye", [P, P], BF16, kind="ExternalInput")
    out_dt = BF16 if out_bf16 else F32
    out_t = nc.dram_tensor("out", [P, nblk * D], out_dt, kind="ExternalOutput")

    with tile.TileContext(nc) as tc, ExitStack() as ctx:
        nc.gpsimd.load_library(mlp_library)
        const = ctx.enter_context(tc.tile_pool(name="const", bufs=1))
        gpool = ctx.enter_context(tc.tile_pool(name="gather", bufs=bufs))
        tpool = ctx.enter_context(tc.tile_pool(name="aggT", bufs=8))
        dpool = ctx.enter_context(tc.tile_pool(name="delta", bufs=bufs))
        psa_bufs, psb_bufs = (psum_bufs if isinstance(psum_bufs, (tuple, list))
                              else (psum_bufs, psum_bufs))
        psA = ctx.enter_context(tc.tile_pool(name="psA", bufs=psa_bufs, space="PSUM"))
        psB = ctx.enter_context(tc.tile_pool(name="psB", bufs=psb_bufs, space="PSUM"))

        ident = const.tile([P, P], BF16)
        nc.sync.dma_start(out=ident[:], in_=eye_t.ap())
        wt_sb = const.tile([D, D], BF16)
        nc.sync.dma_start(out=wt_sb[:], in_=wt_t.ap())
        # two-slice idx load: the first gather only waits for its own slice
        idx_sb = const.tile([P, S * 8], I16)
        g0 = gathers[0][3] * 8
        nc.sync.dma_start(out=idx_sb[:, :g0], in_=idx_t.ap()[:, :g0])
        nc.sync.dma_start(out=idx_sb[:, g0:], in_=idx_t.ap()[:, g0:])
        wgt_sb = const.tile([P, S], BF16)
        nc.sync.dma_start(out=wgt_sb[:], in_=wgt_t.ap())

        # Prime engines on the upfront loads so per-block instructions carry
        # at most one sync wait each (SEQ instruction structs encode one).
        prime = const.tile([P, 1], BF16)
        nc.vector.tensor_copy(out=prime[:], in_=wgt_sb[:, :1])
        prime2 = const.tile([P, 1], BF16)
        nc.scalar.copy(out=prime2[:], in_=ident[:, :1])
        prime_ps = psA.tile([D, P], F32, tag="agg")
        nc.tensor.matmul(out=prime_ps[:], lhsT=ident[:, :D], rhs=ident[:],
                         start=True, stop=True)

        for gi, (blo, bhi, s0, gsz) in enumerate(gathers):
            gt = gpool.tile([P, gsz * D], BF16, tag="g")
            _dma_gather(
                nc.gpsimd,
                gt[:].rearrange("p (c e) -> p c e", e=D),
                tabs[gi].ap(),
                idx_sb[:, s0 * 8 : (s0 + gsz) * 8],
                gsz * P,
            )
            if stages < 2:
                continue
            nc.vector.tensor_tensor(
                out=gt[:].rearrange("p (d o) -> p d o", o=D),
                in0=gt[:].rearrange("p (d o) -> p d o", o=D),
                in1=wgt_sb[:, s0 : s0 + gsz].to_broadcast([P, gsz, D]),
                op=mybir.AluOpType.mult,
            )
            if stages < 3:
                continue

            ng = bhi - blo
            dgrp = dpool.tile([P, ng * D], out_dt, tag="d")
            lo = 0
            for j in range(blo, bhi):
                dj = d_sched[j]
                agg_ps = psA.tile([D, P], F32, tag="agg")
                for dd in range(dj):
                    nc.tensor.matmul(
                        out=agg_ps[:],
                        lhsT=gt[:, (lo + dd) * D : (lo + dd + 1) * D],
                        rhs=ident[:],
                        start=(dd == 0),
                        stop=(dd == dj - 1),
                    )
                aggT = tpool.tile([D, P], BF16, tag="aT")
                if aggT_on_dve:
                    nc.vector.tensor_copy(out=aggT[:], in_=agg_ps[:])
                else:
                    nc.scalar.copy(out=aggT[:], in_=agg_ps[:])
                d_ps = psB.tile([P, D], F32, tag="mm")
                nc.tensor.matmul(out=d_ps[:], lhsT=aggT[:], rhs=wt_sb[:],
                                 start=True, stop=True)
                nc.scalar.copy(out=dgrp[:, (j - blo) * D : (j - blo + 1) * D],
                               in_=d_ps[:])
                lo += dj

            nc.sync.dma_start(out=out_t.ap()[:, blo * D : bhi * D], in_=dgrp[:])
    nc.compile()
    return nc


def run_gcn(x, W, edge_weights, source, target, num_nodes, trace=False, bufs=4):
    """Full-input host entry: preprocess, build, run on 8 cores, assemble output."""
    n_nodes = int(num_nodes)
    pp = preprocess(source, target, edge_weights, n_nodes)
    nc = build_nc(pp, n_nodes, bufs=bufs)
    x_bf = np.ascontiguousarray(np.asarray(x, dtype=np.float32)).astype(NP_BF16)
    wt_np = np.ascontiguousarray(np.asarray(W, dtype=np.float32).T).astype(NP_BF16)
    eye_np = np.eye(P, dtype=np.float32).astype(NP_BF16)
    in_maps = []
    for k in range(N_CORES):
        pc = pp["per_core"][k]
        im = {"wT": wt_np, "eye": eye_np, "idx": pc["idx16"], "wgt": pc["w_all"]}
        for gi, uniq in enumerate(pc["uniq_list"]):
            tab = np.zeros((TABLE_ROWS, 2 * D), dtype=NP_BF16)
            tab[: len(uniq), :D] = x_bf[uniq]
            im[f"xg{gi}"] = tab
        in_maps.append(im)
    res = run_bass_kernel_spmd(nc, in_maps, core_ids=list(range(N_CORES)), trace=trace)

    nt, nblk = pp["nt"], pp["nblk"]
    out = np.empty((n_nodes, D), dtype=np.float32)
    for k in range(N_CORES):
        raw = np.asarray(res.results[k]["out"], dtype=np.float32)  # [128, nblk*64]
        cube = raw.reshape(P, nblk, D).transpose(1, 0, 2).reshape(nblk * P, D)
        perm_pad = pp["per_core"][k]["perm_pad"]
        valid = perm_pad >= 0
        out[k * nt + perm_pad[valid]] = cube[valid]
    return out, res


def kernel(**inputs) -> np.ndarray:
    """Harness entry: full unsharded inputs -> full (num_nodes, 64) output."""
    out, _ = run_gcn(
        np.asarray(inputs["x"]),
        np.asarray(inputs["W"]),
        np.asarray(inputs["edge_weights"]),
        np.asarray(inputs["source"]),
        np.asarray(inputs["target"]),
        int(inputs["num_nodes"]),
        trace=False,
    )
    return out


# revision 28
# speedup vs baseline: 24.2683x; 1.2208x over previous
"""GCN message-passing Bass kernel for TRN2 (8 cores).

Math: delta = segment_sum(w_e * x[src_e]) @ W^T   (linearity: transform after aggregate)

Sharding: targets split across 8 cores (12500 each). Per core, targets are
degree-sorted and grouped into 128-target blocks; block j gives each of its
128 targets D_j padded edge slots (pad -> weight 0). Blocks are packed into
gathers of <= 64 slots (8192 indices).

Per gather: ONE InstDMAGatherAnt pulls bf16 x rows from a per-gather DRAM
table (the unique x rows of that gather's slot entries, host-reindexed to
int16 ids, rows padded to 256B stride with a 128B payload):
  gt[p, s*64:(s+1)*64] = table[idx[s*128+p], :64]
DVE multiplies the gather in-place by per-slot weights (broadcast AP).
PE does the slot reduction: per block, dj accumulating matmuls
(lhsT=slot tile, rhs=identity) sum slot^T into PSUM [64,128]; Act copies the
f32 PSUM to bf16 SBUF; PE applies W^T (aggT as lhsT); Act copies the result
into a per-gather output tile written to DRAM contiguously in block order
(plain DMA, no scatter). The host applies the inverse target permutation to
assemble the final (num_nodes, 64) output.
"""

import math
from contextlib import ExitStack

import numpy as np
import ml_dtypes

import concourse.bass as bass
import concourse.bacc as bacc
import concourse.mybir as mybir
import concourse.tile as tile
from concourse.bass_utils import run_bass_kernel_spmd
from concourse.library_config import mlp as mlp_library

P = 128
N_CORES = 8
D = 64
F32 = mybir.dt.float32
BF16 = mybir.dt.bfloat16
I16 = mybir.dt.int16
NP_BF16 = np.dtype(ml_dtypes.bfloat16)

GATHER_SLOTS = 64  # max slots per dma_gather (64*128 = 8192 indices)
TABLE_ROWS = 8192  # per-gather unique-row table size (int16-indexable)


def preprocess(source, target, edge_weights, n_nodes, n_cores=N_CORES):
    """Build per-core gather tables/indices/weights and the shared schedule.

    Returns dict with:
      d_sched: [nblk] per-block slot count (same for all cores)
      gathers: list of (blo, bhi, s0, gsz) gather groups over blocks
      per_core: dict with tables [G,TABLE_ROWS,128] bf16, idx16 [128,S*8] i16,
                w_all [128,S] bf16, perm_pad [nblk*128] i64
      nt, nblk, S
    """
    source = np.asarray(source).astype(np.int64)
    target = np.asarray(target).astype(np.int64)
    edge_weights = np.asarray(edge_weights).astype(np.float32)
    nt = n_nodes // n_cores
    assert nt * n_cores == n_nodes
    nblk = math.ceil(nt / P)
    ntp = nblk * P

    cores = []
    d_sched = np.zeros(nblk, dtype=np.int64)
    for k in range(n_cores):
        lo, hi = k * nt, (k + 1) * nt
        m = (target >= lo) & (target < hi)
        src_k = source[m]
        w_k = edge_weights[m]
        tl_k = target[m] - lo  # local target ids

        deg = np.bincount(tl_k, minlength=nt)
        perm = np.argsort(deg, kind="stable")  # local ids, degree-ascending
        deg_pad = np.concatenate([deg[perm], np.zeros(ntp - nt, dtype=deg.dtype)])
        d_k = deg_pad.reshape(nblk, P).max(axis=1)
        d_sched = np.maximum(d_sched, d_k)
        cores.append(dict(src=src_k, w=w_k, tl=tl_k, deg=deg, perm=perm))

    offs = np.concatenate([[0], np.cumsum(d_sched)]).astype(np.int64)
    S = int(offs[-1])

    # pack blocks into gathers of <= GATHER_SLOTS slots; keep the first and
    # last few gathers single-block so the pipeline head fill and the
    # un-overlapped tail stay short
    single_tail = 4
    gathers = []
    blo = 0
    while blo < nblk:
        bhi = blo
        gsz = 0
        while (bhi < nblk and (bhi == blo or gsz + d_sched[bhi] <= GATHER_SLOTS)
               and not (bhi > blo and bhi >= nblk - single_tail)):
            gsz += int(d_sched[bhi])
            bhi += 1
        gathers.append((blo, bhi, int(offs[blo]), gsz))
        blo = bhi
    G = len(gathers)

    per_core = []
    for k in range(n_cores):
        c = cores[k]
        deg, perm = c["deg"], c["perm"]
        rank = np.empty(nt, dtype=np.int64)
        rank[perm] = np.arange(nt)

        order = np.argsort(c["tl"], kind="stable")
        tls = c["tl"][order]
        srcs = c["src"][order]
        ws = c["w"][order]
        starts = np.cumsum(deg) - deg  # first edge position per target
        eo = np.arange(len(tls)) - starts[tls]  # occurrence index within target
        rr = rank[tls]
        pp = rr & (P - 1)
        bb = rr >> 7
        col = offs[bb] + eo

        entries = np.zeros((P, S), dtype=np.int64)  # pad -> x row 0 (weight 0)
        w_all = np.zeros((P, S), dtype=NP_BF16)
        entries[pp, col] = srcs
        w_all[pp, col] = ws.astype(NP_BF16)

        # per-gather unique tables + int16 indices, wrapped for the Q7 layout
        uniq_list = []
        idx16 = np.empty((P, S * 8), dtype=np.int16)
        for gi, (_, _, s0, gsz) in enumerate(gathers):
            ent = entries[:, s0 : s0 + gsz]
            uniq, inv = np.unique(ent, return_inverse=True)
            assert len(uniq) <= TABLE_ROWS
            uniq_list.append(uniq)
            inv = inv.reshape(P, gsz).astype(np.int16)
            iflat = inv.T.reshape(-1)  # position i = s_local*128 + p
            blkcols = np.tile(iflat.reshape(gsz * 8, 16).T, (8, 1))
            idx16[:, s0 * 8 : (s0 + gsz) * 8] = blkcols

        perm_pad = np.full(ntp, -1, dtype=np.int64)
        perm_pad[:nt] = perm
        per_core.append(dict(uniq_list=uniq_list, idx16=idx16, w_all=w_all,
                             perm_pad=perm_pad))

    return dict(d_sched=[int(d) for d in d_sched], S=S, gathers=gathers,
                per_core=per_core, nt=nt, nblk=nblk, G=G)


def _dma_gather(gp, out_ap, in_ap, idxs_ap, num_idxs):
    """InstDMAGatherAnt with a 128B payload at 256B row stride (elem_size=64
    bf16, stride_bytes_256=1). bass.dma_gather asserts elem%256B, but the Q7
    ucode handles 128B payloads (verified on HW); construct directly."""
    _in_ap = gp.lower_ap_dma(in_ap, for_custom_bir_dma=True)
    _idxs_ap = gp.lower_ap(idxs_ap)
    _out_ap = gp.lower_ap(out_ap)
    return gp.add_instruction(
        mybir.InstDMAGatherAnt(
            name=gp.bass.get_next_instruction_name(),
            ins=[*_in_ap, _idxs_ap, gp.lower_val_access(gp.to_reg(num_idxs))],
            outs=[_out_ap],
            transpose=False,
            num_idxs=num_idxs,
            elem_size=D,
            stride_bytes_256=1,
            gen_mode=0,
            single_packet=False,
            queue_num=0,
            sbuf_tokens_per_rank=0,
            sbuf_free_dim_per_rank=0,
            sbuf_free_dim_pad_per_rank=0,
            sbuf_byte_offset=0,
        )
    )


def build_nc(pp, n_nodes, bufs=7, out_bf16=True, psum_bufs=4, stages=3,
             aggT_on_dve=False, pair_transpose=False):
    # stages: 1=gather only, 2=+mult, 3=full (ablation knob for timing)
    d_sched, S, nblk, gathers = pp["d_sched"], pp["S"], pp["nblk"], pp["gathers"]
    nc = bacc.Bacc("TRN2", target_bir_lowering=False, debug=False)
    tabs = [nc.dram_tensor(f"xg{gi}", [TABLE_ROWS, 2 * D], BF16, kind="ExternalInput")
            for gi in range(len(gathers))]
    wt_t = nc.dram_tensor("wT", [D, D], BF16, kind="ExternalInput")
    idx_t = nc.dram_tensor("idx", [P, S * 8], I16, kind="ExternalInput")
    wgt_t = nc.dram_tensor("wgt", [P, S], BF16, kind="ExternalInput")
    eye_t = nc.dram_tensor("eye", [P, P], BF16, kind="ExternalInput")
    out_dt = BF16 if out_bf16 else F32
    out_t = nc.dram_tensor("out", [P, nblk * D], out_dt, kind="ExternalOutput")

    with tile.TileContext(nc) as tc, ExitStack() as ctx:
        nc.gpsimd.load_library(mlp_library)
        const = ctx.enter_context(tc.tile_pool(name="const", bufs=1))
        gpool = ctx.enter_context(tc.tile_pool(name="gather", bufs=bufs))
        tpool = ctx.enter_context(tc.tile_pool(name="aggT", bufs=8))
        dpool = ctx.enter_context(tc.tile_pool(name="delta", bufs=bufs))
        psa_bufs, psb_bufs = (psum_bufs if isinstance(psum_bufs, (tuple, list))
                              else (psum_bufs, psum_bufs))
        psA = ctx.enter_context(tc.tile_pool(name="psA", bufs=psa_bufs, space="PSUM"))
        psB = ctx.enter_context(tc.tile_pool(name="psB", bufs=psb_bufs, space="PSUM"))

        ident = const.tile([P, P], BF16)
        nc.sync.dma_start(out=ident[:], in_=eye_t.ap())
        wt_sb = const.tile([D, D], BF16)
        nc.sync.dma_start(out=wt_sb[:], in_=wt_t.ap())
        # two-slice idx load: the first gather only waits for its own slice
        idx_sb = const.tile([P, S * 8], I16)
        g0 = gathers[0][3] * 8
        nc.sync.dma_start(out=idx_sb[:, :g0], in_=idx_t.ap()[:, :g0])
        nc.sync.dma_start(out=idx_sb[:, g0:], in_=idx_t.ap()[:, g0:])
        wgt_sb = const.tile([P, S], BF16)
        nc.sync.dma_start(out=wgt_sb[:], in_=wgt_t.ap())

        # Prime engines on the upfront loads so per-block instructions carry
        # at most one sync wait each (SEQ instruction structs encode one).
        prime = const.tile([P, 1], BF16)
        nc.vector.tensor_copy(out=prime[:], in_=wgt_sb[:, :1])
        prime2 = const.tile([P, 1], BF16)
        nc.scalar.copy(out=prime2[:], in_=ident[:, :1])
        prime_ps = psA.tile([D, P], F32, tag="agg")
        nc.tensor.matmul(out=prime_ps[:], lhsT=ident[:, :D], rhs=ident[:],
                         start=True, stop=True)

        for gi, (blo, bhi, s0, gsz) in enumerate(gathers):
            gt = gpool.tile([P, gsz * D], BF16, tag="g")
            _dma_gather(
                nc.gpsimd,
                gt[:].rearrange("p (c e) -> p c e", e=D),
                tabs[gi].ap(),
                idx_sb[:, s0 * 8 : (s0 + gsz) * 8],
                gsz * P,
            )
            if stages < 2:
                continue
            nc.vector.tensor_tensor(
                out=gt[:].rearrange("p (d o) -> p d o", o=D),
                in0=gt[:].rearrange("p (d o) -> p d o", o=D),
                in1=wgt_sb[:, s0 : s0 + gsz].to_broadcast([P, gsz, D]),
                op=mybir.AluOpType.mult,
            )
            if stages < 3:
                continue

            ng = bhi - blo
            dgrp = dpool.tile([P, ng * D], out_dt, tag="d")
            lo = 0
            for j in range(blo, bhi):
                dj = d_sched[j]
                aggT = tpool.tile([D, P], BF16, tag="aT")
                if pair_transpose:
                    # two slots per matmul: psum rows 0:64 = even-slot sum^T,
                    # rows 64:128 = odd-slot sum^T; DVE adds the halves
                    agg_ps = psA.tile([P, P], F32, tag="agg")
                    npair = (dj + 1) // 2
                    for i in range(npair):
                        w = 2 if 2 * i + 1 < dj else 1
                        nc.tensor.matmul(
                            out=agg_ps[: w * D, :],
                            lhsT=gt[:, (lo + 2 * i) * D : (lo + 2 * i + w) * D],
                            rhs=ident[:],
                            start=(i == 0),
                            stop=(i == npair - 1),
                        )
                    if dj >= 2:
                        nc.vector.tensor_tensor(
                            out=aggT[:], in0=agg_ps[:D, :], in1=agg_ps[D:, :],
                            op=mybir.AluOpType.add)
                    else:
                        nc.vector.tensor_copy(out=aggT[:], in_=agg_ps[:D, :])
                else:
                    agg_ps = psA.tile([D, P], F32, tag="agg")
                    for dd in range(dj):
                        nc.tensor.matmul(
                            out=agg_ps[:],
                            lhsT=gt[:, (lo + dd) * D : (lo + dd + 1) * D],
                            rhs=ident[:],
                            start=(dd == 0),
                            stop=(dd == dj - 1),
                        )
                    if aggT_on_dve:
                        nc.vector.tensor_copy(out=aggT[:], in_=agg_ps[:])
                    else:
                        nc.scalar.copy(out=aggT[:], in_=agg_ps[:])
                d_ps = psB.tile([P, D], F32, tag="mm")
                nc.tensor.matmul(out=d_ps[:], lhsT=aggT[:], rhs=wt_sb[:],
                                 start=True, stop=True)
                nc.scalar.copy(out=dgrp[:, (j - blo) * D : (j - blo + 1) * D],
                               in_=d_ps[:])
                lo += dj

            nc.sync.dma_start(out=out_t.ap()[:, blo * D : bhi * D], in_=dgrp[:])
    nc.compile()
    return nc


def run_gcn(x, W, edge_weights, source, target, num_nodes, trace=False, bufs=7):
    """Full-input host entry: preprocess, build, run on 8 cores, assemble output."""
    n_nodes = int(num_nodes)
    pp = preprocess(source, target, edge_weights, n_nodes)
    nc = build_nc(pp, n_nodes, bufs=bufs)
    x_bf = np.ascontiguousarray(np.asarray(x, dtype=np.float32)).astype(NP_BF16)
    wt_np = np.ascontiguousarray(np.asarray(W, dtype=np.float32).T).astype(NP_BF16)
    eye_np = np.eye(P, dtype=np.float32).astype(NP_BF16)
    in_maps = []
    for k in range(N_CORES):
        pc = pp["per_core"][k]
        im = {"wT": wt_np, "eye": eye_np, "idx": pc["idx16"], "wgt": pc["w_all"]}
        for gi, uniq in enumerate(pc["uniq_list"]):
            tab = np.zeros((TABLE_ROWS, 2 * D), dtype=NP_BF16)
            tab[: len(uniq), :D] = x_bf[uniq]
            im[f"xg{gi}"] = tab
        in_maps.append(im)
    res = run_bass_kernel_spmd(nc, in_maps, core_ids=list(range(N_CORES)), trace=trace)

    nt, nblk = pp["nt"], pp["nblk"]
    out = np.empty((n_nodes, D), dtype=np.float32)
    for k in range(N_CORES):
        raw = np.asarray(res.results[k]["out"], dtype=np.float32)  # [128, nblk*64]
        cube = raw.reshape(P, nblk, D).transpose(1, 0, 2).reshape(nblk * P, D)
        perm_pad = pp["per_core"][k]["perm_pad"]
        valid = perm_pad >= 0
        out[k * nt + perm_pad[valid]] = cube[valid]
    return out, res


def kernel(**inputs) -> np.ndarray:
    """Harness entry: full unsharded inputs -> full (num_nodes, 64) output."""
    out, _ = run_gcn(
        np.asarray(inputs["x"]),
        np.asarray(inputs["W"]),
        np.asarray(inputs["edge_weights"]),
        np.asarray(inputs["source"]),
        np.asarray(inputs["target"]),
        int(inputs["num_nodes"]),
        trace=False,
    )
    return out


# revision 29
# speedup vs baseline: 24.4791x; 1.0087x over previous
"""GCN message-passing Bass kernel for TRN2 (8 cores).

Math: delta = segment_sum(w_e * x[src_e]) @ W^T   (linearity: transform after aggregate)

Sharding: targets split across 8 cores (12500 each). Per core, targets are
degree-sorted and grouped into 128-target blocks; block j gives each of its
128 targets D_j padded edge slots (pad -> weight 0). Blocks are packed into
gathers of <= 64 slots (8192 indices).

Per gather: ONE InstDMAGatherAnt pulls bf16 x rows from a per-gather DRAM
table (the unique x rows of that gather's slot entries, host-reindexed to
int16 ids, rows padded to 256B stride with a 128B payload):
  gt[p, s*64:(s+1)*64] = table[idx[s*128+p], :64]
DVE multiplies the gather in-place by per-slot weights (broadcast AP).
PE does the slot reduction: per block, dj accumulating matmuls
(lhsT=slot tile, rhs=identity) sum slot^T into PSUM [64,128]; Act copies the
f32 PSUM to bf16 SBUF; PE applies W^T (aggT as lhsT); Act copies the result
into a per-gather output tile written to DRAM contiguously in block order
(plain DMA, no scatter). The host applies the inverse target permutation to
assemble the final (num_nodes, 64) output.
"""

import math
from contextlib import ExitStack

import numpy as np
import ml_dtypes

import concourse.bass as bass
import concourse.bacc as bacc
import concourse.mybir as mybir
import concourse.tile as tile
from concourse.bass_utils import run_bass_kernel_spmd
from concourse.library_config import mlp as mlp_library

P = 128
N_CORES = 8
D = 64
F32 = mybir.dt.float32
BF16 = mybir.dt.bfloat16
I16 = mybir.dt.int16
NP_BF16 = np.dtype(ml_dtypes.bfloat16)

GATHER_SLOTS = 48  # max slots per dma_gather (48*128 = 6144 indices)
TABLE_ROWS = 6144  # per-gather unique-row table size (int16-indexable)


def preprocess(source, target, edge_weights, n_nodes, n_cores=N_CORES):
    """Build per-core gather tables/indices/weights and the shared schedule.

    Returns dict with:
      d_sched: [nblk] per-block slot count (same for all cores)
      gathers: list of (blo, bhi, s0, gsz) gather groups over blocks
      per_core: dict with tables [G,TABLE_ROWS,128] bf16, idx16 [128,S*8] i16,
                w_all [128,S] bf16, perm_pad [nblk*128] i64
      nt, nblk, S
    """
    source = np.asarray(source).astype(np.int64)
    target = np.asarray(target).astype(np.int64)
    edge_weights = np.asarray(edge_weights).astype(np.float32)
    nt = n_nodes // n_cores
    assert nt * n_cores == n_nodes
    nblk = math.ceil(nt / P)
    ntp = nblk * P

    cores = []
    d_sched = np.zeros(nblk, dtype=np.int64)
    for k in range(n_cores):
        lo, hi = k * nt, (k + 1) * nt
        m = (target >= lo) & (target < hi)
        src_k = source[m]
        w_k = edge_weights[m]
        tl_k = target[m] - lo  # local target ids

        deg = np.bincount(tl_k, minlength=nt)
        perm = np.argsort(deg, kind="stable")  # local ids, degree-ascending
        deg_pad = np.concatenate([deg[perm], np.zeros(ntp - nt, dtype=deg.dtype)])
        d_k = deg_pad.reshape(nblk, P).max(axis=1)
        d_sched = np.maximum(d_sched, d_k)
        cores.append(dict(src=src_k, w=w_k, tl=tl_k, deg=deg, perm=perm))

    offs = np.concatenate([[0], np.cumsum(d_sched)]).astype(np.int64)
    S = int(offs[-1])

    # pack blocks into gathers of <= GATHER_SLOTS slots; keep the first and
    # last few gathers single-block so the pipeline head fill and the
    # un-overlapped tail stay short
    single_tail = 4
    gathers = []
    blo = 0
    while blo < nblk:
        bhi = blo
        gsz = 0
        while (bhi < nblk and (bhi == blo or gsz + d_sched[bhi] <= GATHER_SLOTS)
               and not (bhi > blo and bhi >= nblk - single_tail)):
            gsz += int(d_sched[bhi])
            bhi += 1
        gathers.append((blo, bhi, int(offs[blo]), gsz))
        blo = bhi
    G = len(gathers)

    per_core = []
    for k in range(n_cores):
        c = cores[k]
        deg, perm = c["deg"], c["perm"]
        rank = np.empty(nt, dtype=np.int64)
        rank[perm] = np.arange(nt)

        order = np.argsort(c["tl"], kind="stable")
        tls = c["tl"][order]
        srcs = c["src"][order]
        ws = c["w"][order]
        starts = np.cumsum(deg) - deg  # first edge position per target
        eo = np.arange(len(tls)) - starts[tls]  # occurrence index within target
        rr = rank[tls]
        pp = rr & (P - 1)
        bb = rr >> 7
        col = offs[bb] + eo

        entries = np.zeros((P, S), dtype=np.int64)  # pad -> x row 0 (weight 0)
        w_all = np.zeros((P, S), dtype=NP_BF16)
        entries[pp, col] = srcs
        w_all[pp, col] = ws.astype(NP_BF16)

        # per-gather unique tables + int16 indices, wrapped for the Q7 layout
        uniq_list = []
        idx16 = np.empty((P, S * 8), dtype=np.int16)
        for gi, (_, _, s0, gsz) in enumerate(gathers):
            ent = entries[:, s0 : s0 + gsz]
            uniq, inv = np.unique(ent, return_inverse=True)
            assert len(uniq) <= TABLE_ROWS
            uniq_list.append(uniq)
            inv = inv.reshape(P, gsz).astype(np.int16)
            iflat = inv.T.reshape(-1)  # position i = s_local*128 + p
            blkcols = np.tile(iflat.reshape(gsz * 8, 16).T, (8, 1))
            idx16[:, s0 * 8 : (s0 + gsz) * 8] = blkcols

        perm_pad = np.full(ntp, -1, dtype=np.int64)
        perm_pad[:nt] = perm
        per_core.append(dict(uniq_list=uniq_list, idx16=idx16, w_all=w_all,
                             perm_pad=perm_pad))

    return dict(d_sched=[int(d) for d in d_sched], S=S, gathers=gathers,
                per_core=per_core, nt=nt, nblk=nblk, G=G)


def _dma_gather(gp, out_ap, in_ap, idxs_ap, num_idxs):
    """InstDMAGatherAnt with a 128B payload at 256B row stride (elem_size=64
    bf16, stride_bytes_256=1). bass.dma_gather asserts elem%256B, but the Q7
    ucode handles 128B payloads (verified on HW); construct directly."""
    _in_ap = gp.lower_ap_dma(in_ap, for_custom_bir_dma=True)
    _idxs_ap = gp.lower_ap(idxs_ap)
    _out_ap = gp.lower_ap(out_ap)
    return gp.add_instruction(
        mybir.InstDMAGatherAnt(
            name=gp.bass.get_next_instruction_name(),
            ins=[*_in_ap, _idxs_ap, gp.lower_val_access(gp.to_reg(num_idxs))],
            outs=[_out_ap],
            transpose=False,
            num_idxs=num_idxs,
            elem_size=D,
            stride_bytes_256=1,
            gen_mode=0,
            single_packet=False,
            queue_num=0,
            sbuf_tokens_per_rank=0,
            sbuf_free_dim_per_rank=0,
            sbuf_free_dim_pad_per_rank=0,
            sbuf_byte_offset=0,
        )
    )


def build_nc(pp, n_nodes, bufs=9, out_bf16=True, psum_bufs=4, stages=3,
             aggT_on_dve=False, pair_transpose=False):
    # stages: 1=gather only, 2=+mult, 3=full (ablation knob for timing)
    d_sched, S, nblk, gathers = pp["d_sched"], pp["S"], pp["nblk"], pp["gathers"]
    nc = bacc.Bacc("TRN2", target_bir_lowering=False, debug=False)
    tabs = [nc.dram_tensor(f"xg{gi}", [TABLE_ROWS, 2 * D], BF16, kind="ExternalInput")
            for gi in range(len(gathers))]
    wt_t = nc.dram_tensor("wT", [D, D], BF16, kind="ExternalInput")
    idx_t = nc.dram_tensor("idx", [P, S * 8], I16, kind="ExternalInput")
    wgt_t = nc.dram_tensor("wgt", [P, S], BF16, kind="ExternalInput")
    eye_t = nc.dram_tensor("eye", [P, P], BF16, kind="ExternalInput")
    out_dt = BF16 if out_bf16 else F32
    out_t = nc.dram_tensor("out", [P, nblk * D], out_dt, kind="ExternalOutput")

    with tile.TileContext(nc) as tc, ExitStack() as ctx:
        nc.gpsimd.load_library(mlp_library)
        const = ctx.enter_context(tc.tile_pool(name="const", bufs=1))
        gpool = ctx.enter_context(tc.tile_pool(name="gather", bufs=bufs))
        tpool = ctx.enter_context(tc.tile_pool(name="aggT", bufs=8))
        dpool = ctx.enter_context(tc.tile_pool(name="delta", bufs=bufs))
        psa_bufs, psb_bufs = (psum_bufs if isinstance(psum_bufs, (tuple, list))
                              else (psum_bufs, psum_bufs))
        psA = ctx.enter_context(tc.tile_pool(name="psA", bufs=psa_bufs, space="PSUM"))
        psB = ctx.enter_context(tc.tile_pool(name="psB", bufs=psb_bufs, space="PSUM"))

        ident = const.tile([P, P], BF16)
        nc.sync.dma_start(out=ident[:], in_=eye_t.ap())
        wt_sb = const.tile([D, D], BF16)
        nc.sync.dma_start(out=wt_sb[:], in_=wt_t.ap())
        # two-slice idx load: the first gather only waits for its own slice
        idx_sb = const.tile([P, S * 8], I16)
        g0 = gathers[0][3] * 8
        nc.sync.dma_start(out=idx_sb[:, :g0], in_=idx_t.ap()[:, :g0])
        nc.sync.dma_start(out=idx_sb[:, g0:], in_=idx_t.ap()[:, g0:])
        wgt_sb = const.tile([P, S], BF16)
        nc.sync.dma_start(out=wgt_sb[:], in_=wgt_t.ap())

        # Prime engines on the upfront loads so per-block instructions carry
        # at most one sync wait each (SEQ instruction structs encode one).
        prime = const.tile([P, 1], BF16)
        nc.vector.tensor_copy(out=prime[:], in_=wgt_sb[:, :1])
        prime2 = const.tile([P, 1], BF16)
        nc.scalar.copy(out=prime2[:], in_=ident[:, :1])
        prime_ps = psA.tile([D, P], F32, tag="agg")
        nc.tensor.matmul(out=prime_ps[:], lhsT=ident[:, :D], rhs=ident[:],
                         start=True, stop=True)

        for gi, (blo, bhi, s0, gsz) in enumerate(gathers):
            gt = gpool.tile([P, gsz * D], BF16, tag="g")
            _dma_gather(
                nc.gpsimd,
                gt[:].rearrange("p (c e) -> p c e", e=D),
                tabs[gi].ap(),
                idx_sb[:, s0 * 8 : (s0 + gsz) * 8],
                gsz * P,
            )
            if stages < 2:
                continue
            nc.vector.tensor_tensor(
                out=gt[:].rearrange("p (d o) -> p d o", o=D),
                in0=gt[:].rearrange("p (d o) -> p d o", o=D),
                in1=wgt_sb[:, s0 : s0 + gsz].to_broadcast([P, gsz, D]),
                op=mybir.AluOpType.mult,
            )
            if stages < 3:
                continue

            ng = bhi - blo
            dgrp = dpool.tile([P, ng * D], out_dt, tag="d")
            lo = 0
            for j in range(blo, bhi):
                dj = d_sched[j]
                aggT = tpool.tile([D, P], BF16, tag="aT")
                if pair_transpose:
                    # two slots per matmul: psum rows 0:64 = even-slot sum^T,
                    # rows 64:128 = odd-slot sum^T; DVE adds the halves
                    agg_ps = psA.tile([P, P], F32, tag="agg")
                    npair = (dj + 1) // 2
                    for i in range(npair):
                        w = 2 if 2 * i + 1 < dj else 1
                        nc.tensor.matmul(
                            out=agg_ps[: w * D, :],
                            lhsT=gt[:, (lo + 2 * i) * D : (lo + 2 * i + w) * D],
                            rhs=ident[:],
                            start=(i == 0),
                            stop=(i == npair - 1),
                        )
                    if dj >= 2:
                        nc.vector.tensor_tensor(
                            out=aggT[:], in0=agg_ps[:D, :], in1=agg_ps[D:, :],
                            op=mybir.AluOpType.add)
                    else:
                        nc.vector.tensor_copy(out=aggT[:], in_=agg_ps[:D, :])
                else:
                    agg_ps = psA.tile([D, P], F32, tag="agg")
                    for dd in range(dj):
                        nc.tensor.matmul(
                            out=agg_ps[:],
                            lhsT=gt[:, (lo + dd) * D : (lo + dd + 1) * D],
                            rhs=ident[:],
                            start=(dd == 0),
                            stop=(dd == dj - 1),
                        )
                    if aggT_on_dve:
                        nc.vector.tensor_copy(out=aggT[:], in_=agg_ps[:])
                    else:
                        nc.scalar.copy(out=aggT[:], in_=agg_ps[:])
                d_ps = psB.tile([P, D], F32, tag="mm")
                nc.tensor.matmul(out=d_ps[:], lhsT=aggT[:], rhs=wt_sb[:],
                                 start=True, stop=True)
                nc.scalar.copy(out=dgrp[:, (j - blo) * D : (j - blo + 1) * D],
                               in_=d_ps[:])
                lo += dj

            nc.sync.dma_start(out=out_t.ap()[:, blo * D : bhi * D], in_=dgrp[:])
    nc.compile()
    return nc


def run_gcn(x, W, edge_weights, source, target, num_nodes, trace=False, bufs=9):
    """Full-input host entry: preprocess, build, run on 8 cores, assemble output."""
    n_nodes = int(num_nodes)
    pp = preprocess(source, target, edge_weights, n_nodes)
    nc = build_nc(pp, n_nodes, bufs=bufs)
    x_bf = np.ascontiguousarray(np.asarray(x, dtype=np.float32)).astype(NP_BF16)
    wt_np = np.ascontiguousarray(np.asarray(W, dtype=np.float32).T).astype(NP_BF16)
    eye_np = np.eye(P, dtype=np.float32).astype(NP_BF16)
    in_maps = []
    for k in range(N_CORES):
        pc = pp["per_core"][k]
        im = {"wT": wt_np, "eye": eye_np, "idx": pc["idx16"], "wgt": pc["w_all"]}
        for gi, uniq in enumerate(pc["uniq_list"]):
            tab = np.zeros((TABLE_ROWS, 2 * D), dtype=NP_BF16)
            tab[: len(uniq), :D] = x_bf[uniq]
            im[f"xg{gi}"] = tab
        in_maps.append(im)
    res = run_bass_kernel_spmd(nc, in_maps, core_ids=list(range(N_CORES)), trace=trace)

    nt, nblk = pp["nt"], pp["nblk"]
    out = np.empty((n_nodes, D), dtype=np.float32)
    for k in range(N_CORES):
        raw = np.asarray(res.results[k]["out"], dtype=np.float32)  # [128, nblk*64]
        cube = raw.reshape(P, nblk, D).transpose(1, 0, 2).reshape(nblk * P, D)
        perm_pad = pp["per_core"][k]["perm_pad"]
        valid = perm_pad >= 0
        out[k * nt + perm_pad[valid]] = cube[valid]
    return out, res


def kernel(**inputs) -> np.ndarray:
    """Harness entry: full unsharded inputs -> full (num_nodes, 64) output."""
    out, _ = run_gcn(
        np.asarray(inputs["x"]),
        np.asarray(inputs["W"]),
        np.asarray(inputs["edge_weights"]),
        np.asarray(inputs["source"]),
        np.asarray(inputs["target"]),
        int(inputs["num_nodes"]),
        trace=False,
    )
    return out


# revision 37
# speedup vs baseline: 27.6629x; 1.1301x over previous
"""GCN message-passing Bass kernel for TRN2 (8 cores).

Math: delta = segment_sum(w_e * x[src_e]) @ W^T   (linearity: transform after aggregate)

Sharding: targets split across 8 cores (12500 each). Per core, targets are
degree-sorted and grouped into 128-target blocks; block j gives each of its
128 targets D_j padded edge slots (pad -> weight 0). Blocks are packed into
gathers of <= 64 slots (8192 indices).

Per gather: ONE InstDMAGatherAnt pulls bf16 x rows from a per-gather DRAM
table (the unique x rows of that gather's slot entries, host-reindexed to
int16 ids, rows padded to 256B stride with a 128B payload):
  gt[p, s*64:(s+1)*64] = table[idx[s*128+p], :64]
DVE multiplies the gather in-place by per-slot weights (broadcast AP).
PE does the slot reduction: per block, dj accumulating matmuls
(lhsT=slot tile, rhs=identity) sum slot^T into PSUM [64,128]; Act copies the
f32 PSUM to bf16 SBUF; PE applies W^T (aggT as lhsT); Act copies the result
into a per-gather output tile written to DRAM contiguously in block order
(plain DMA, no scatter). The host applies the inverse target permutation to
assemble the final (num_nodes, 64) output.
"""

import math
from contextlib import ExitStack

import numpy as np
import ml_dtypes

import concourse.bass as bass
import concourse.bacc as bacc
import concourse.mybir as mybir
import concourse.tile as tile
from concourse.bass_utils import run_bass_kernel_spmd
from concourse.library_config import mlp as mlp_library

P = 128
N_CORES = 8
D = 64
F32 = mybir.dt.float32
BF16 = mybir.dt.bfloat16
I16 = mybir.dt.int16
I8 = mybir.dt.int8
NP_BF16 = np.dtype(ml_dtypes.bfloat16)

GATHER_SLOTS = 64  # max slots per dma_gather (64*128 = 8192 indices)
TABLE_ROWS = 8192  # per-gather unique-row table size (int16-indexable)


def preprocess(source, target, edge_weights, n_nodes, n_cores=N_CORES,
               src_scale=None):
    """Build per-core gather tables/indices/weights and the shared schedule.

    Returns dict with:
      d_sched: [nblk] per-block slot count (same for all cores)
      gathers: list of (blo, bhi, s0, gsz) gather groups over blocks
      per_core: dict with tables [G,TABLE_ROWS,128] bf16, idx16 [128,S*8] i16,
                w_all [128,S] bf16, perm_pad [nblk*128] i64
      nt, nblk, S
    """
    source = np.asarray(source).astype(np.int64)
    target = np.asarray(target).astype(np.int64)
    edge_weights = np.asarray(edge_weights).astype(np.float32)
    nt = n_nodes // n_cores
    assert nt * n_cores == n_nodes
    nblk = math.ceil(nt / P)
    ntp = nblk * P

    cores = []
    d_sched = np.zeros(nblk, dtype=np.int64)
    for k in range(n_cores):
        lo, hi = k * nt, (k + 1) * nt
        m = (target >= lo) & (target < hi)
        src_k = source[m]
        w_k = edge_weights[m]
        tl_k = target[m] - lo  # local target ids

        deg = np.bincount(tl_k, minlength=nt)
        perm = np.argsort(deg, kind="stable")  # local ids, degree-ascending
        deg_pad = np.concatenate([deg[perm], np.zeros(ntp - nt, dtype=deg.dtype)])
        d_k = deg_pad.reshape(nblk, P).max(axis=1)
        d_sched = np.maximum(d_sched, d_k)
        cores.append(dict(src=src_k, w=w_k, tl=tl_k, deg=deg, perm=perm))

    offs = np.concatenate([[0], np.cumsum(d_sched)]).astype(np.int64)
    S = int(offs[-1])

    # pack blocks into gathers of <= GATHER_SLOTS slots; keep the first and
    # last few gathers single-block so the pipeline head fill and the
    # un-overlapped tail stay short
    single_tail = 4
    gathers = []
    blo = 0
    while blo < nblk:
        bhi = blo
        gsz = 0
        while (bhi < nblk and (bhi == blo or gsz + d_sched[bhi] <= GATHER_SLOTS)
               and not (bhi > blo and bhi >= nblk - single_tail)):
            gsz += int(d_sched[bhi])
            bhi += 1
        gathers.append((blo, bhi, int(offs[blo]), gsz))
        blo = bhi
    G = len(gathers)

    per_core = []
    for k in range(n_cores):
        c = cores[k]
        deg, perm = c["deg"], c["perm"]
        rank = np.empty(nt, dtype=np.int64)
        rank[perm] = np.arange(nt)

        order = np.argsort(c["tl"], kind="stable")
        tls = c["tl"][order]
        srcs = c["src"][order]
        ws = c["w"][order]
        starts = np.cumsum(deg) - deg  # first edge position per target
        eo = np.arange(len(tls)) - starts[tls]  # occurrence index within target
        rr = rank[tls]
        pp = rr & (P - 1)
        bb = rr >> 7
        col = offs[bb] + eo

        entries = np.zeros((P, S), dtype=np.int64)  # pad -> x row 0 (weight 0)
        w_all = np.zeros((P, S), dtype=NP_BF16)
        entries[pp, col] = srcs
        # fold the int8 per-source-row dequant scale into the edge weight
        wsf = ws if src_scale is None else ws * src_scale[srcs].astype(np.float32)
        w_all[pp, col] = wsf.astype(NP_BF16)

        # per-gather unique tables + int16 indices, wrapped for the Q7 layout
        uniq_list = []
        idx16 = np.empty((P, S * 8), dtype=np.int16)
        for gi, (_, _, s0, gsz) in enumerate(gathers):
            ent = entries[:, s0 : s0 + gsz]
            uniq, inv = np.unique(ent, return_inverse=True)
            assert len(uniq) <= TABLE_ROWS
            uniq_list.append(uniq)
            inv = inv.reshape(P, gsz).astype(np.int16)
            iflat = inv.T.reshape(-1)  # position i = s_local*128 + p
            blkcols = np.tile(iflat.reshape(gsz * 8, 16).T, (8, 1))
            idx16[:, s0 * 8 : (s0 + gsz) * 8] = blkcols

        perm_pad = np.full(ntp, -1, dtype=np.int64)
        perm_pad[:nt] = perm
        per_core.append(dict(uniq_list=uniq_list, idx16=idx16, w_all=w_all,
                             perm_pad=perm_pad))

    return dict(d_sched=[int(d) for d in d_sched], S=S, gathers=gathers,
                per_core=per_core, nt=nt, nblk=nblk, G=G)


def _dma_gather(gp, out_ap, in_ap, idxs_ap, num_idxs):
    """InstDMAGatherAnt with a 128B payload at 256B row stride (elem_size=64
    bf16, stride_bytes_256=1). bass.dma_gather asserts elem%256B, but the Q7
    ucode handles 128B payloads (verified on HW); construct directly."""
    _in_ap = gp.lower_ap_dma(in_ap, for_custom_bir_dma=True)
    _idxs_ap = gp.lower_ap(idxs_ap)
    _out_ap = gp.lower_ap(out_ap)
    return gp.add_instruction(
        mybir.InstDMAGatherAnt(
            name=gp.bass.get_next_instruction_name(),
            ins=[*_in_ap, _idxs_ap, gp.lower_val_access(gp.to_reg(num_idxs))],
            outs=[_out_ap],
            transpose=False,
            num_idxs=num_idxs,
            elem_size=D,
            stride_bytes_256=1,
            gen_mode=0,
            single_packet=False,
            queue_num=0,
            sbuf_tokens_per_rank=0,
            sbuf_free_dim_per_rank=0,
            sbuf_free_dim_pad_per_rank=0,
            sbuf_byte_offset=0,
        )
    )


def build_nc(pp, n_nodes, bufs=9, out_bf16=True, psum_bufs=4, stages=3,
             aggT_on_dve=False, pair_transpose=False):
    # stages: 1=gather only, 2=+mult, 3=full (ablation knob for timing)
    d_sched, S, nblk, gathers = pp["d_sched"], pp["S"], pp["nblk"], pp["gathers"]
    nc = bacc.Bacc("TRN2", target_bir_lowering=False, debug=False)
    # int8 rows, padded to a 256B stride; payload = first 64 bytes
    tabs = [nc.dram_tensor(f"xg{gi}", [TABLE_ROWS, 4 * D], I8, kind="ExternalInput")
            for gi in range(len(gathers))]
    wt_t = nc.dram_tensor("wT", [D, D], BF16, kind="ExternalInput")
    idx_t = nc.dram_tensor("idx", [P, S * 8], I16, kind="ExternalInput")
    wgt_t = nc.dram_tensor("wgt", [P, S], BF16, kind="ExternalInput")
    eye_t = nc.dram_tensor("eye", [P, P], BF16, kind="ExternalInput")
    out_dt = BF16 if out_bf16 else F32
    out_t = nc.dram_tensor("out", [P, nblk * D], out_dt, kind="ExternalOutput")

    with tile.TileContext(nc) as tc, ExitStack() as ctx:
        nc.gpsimd.load_library(mlp_library)
        const = ctx.enter_context(tc.tile_pool(name="const", bufs=1))
        gpool = ctx.enter_context(tc.tile_pool(name="gather", bufs=bufs))
        mpool = ctx.enter_context(tc.tile_pool(name="msg", bufs=bufs))
        tpool = ctx.enter_context(tc.tile_pool(name="aggT", bufs=8))
        dpool = ctx.enter_context(tc.tile_pool(name="delta", bufs=bufs))
        psa_bufs, psb_bufs = (psum_bufs if isinstance(psum_bufs, (tuple, list))
                              else (psum_bufs, psum_bufs))
        psA = ctx.enter_context(tc.tile_pool(name="psA", bufs=psa_bufs, space="PSUM"))
        psB = ctx.enter_context(tc.tile_pool(name="psB", bufs=psb_bufs, space="PSUM"))

        ident = const.tile([P, P], BF16)
        nc.sync.dma_start(out=ident[:], in_=eye_t.ap())
        wt_sb = const.tile([D, D], BF16)
        nc.sync.dma_start(out=wt_sb[:], in_=wt_t.ap())
        # two-slice idx load: the first gather only waits for its own slice
        idx_sb = const.tile([P, S * 8], I16)
        g0 = gathers[0][3] * 8
        nc.sync.dma_start(out=idx_sb[:, :g0], in_=idx_t.ap()[:, :g0])
        nc.sync.dma_start(out=idx_sb[:, g0:], in_=idx_t.ap()[:, g0:])
        wgt_sb = const.tile([P, S], BF16)
        nc.sync.dma_start(out=wgt_sb[:], in_=wgt_t.ap())

        # Prime engines on the upfront loads so per-block instructions carry
        # at most one sync wait each (SEQ instruction structs encode one).
        prime = const.tile([P, 1], BF16)
        nc.vector.tensor_copy(out=prime[:], in_=wgt_sb[:, :1])
        prime2 = const.tile([P, 1], BF16)
        nc.scalar.copy(out=prime2[:], in_=ident[:, :1])
        prime_ps = psA.tile([D, P], F32, tag="agg")
        nc.tensor.matmul(out=prime_ps[:], lhsT=ident[:, :D], rhs=ident[:],
                         start=True, stop=True)

        for gi, (blo, bhi, s0, gsz) in enumerate(gathers):
            gt = gpool.tile([P, gsz * D], I8, tag="g")
            _dma_gather(
                nc.gpsimd,
                gt[:].rearrange("p (c e) -> p c e", e=D),
                tabs[gi].ap(),
                idx_sb[:, s0 * 8 : (s0 + gsz) * 8],
                gsz * P,
            )
            if stages < 2:
                continue
            msg = mpool.tile([P, gsz * D], BF16, tag="m")
            nc.vector.tensor_tensor(
                out=msg[:].rearrange("p (d o) -> p d o", o=D),
                in0=gt[:].rearrange("p (d o) -> p d o", o=D),
                in1=wgt_sb[:, s0 : s0 + gsz].to_broadcast([P, gsz, D]),
                op=mybir.AluOpType.mult,
            )
            if stages < 3:
                continue

            ng = bhi - blo
            dgrp = dpool.tile([P, ng * D], out_dt, tag="d")
            lo = 0
            for j in range(blo, bhi):
                dj = d_sched[j]
                aggT = tpool.tile([D, P], BF16, tag="aT")
                if pair_transpose:
                    # two slots per matmul: psum rows 0:64 = even-slot sum^T,
                    # rows 64:128 = odd-slot sum^T; DVE adds the halves
                    agg_ps = psA.tile([P, P], F32, tag="agg")
                    npair = (dj + 1) // 2
                    for i in range(npair):
                        w = 2 if 2 * i + 1 < dj else 1
                        nc.tensor.matmul(
                            out=agg_ps[: w * D, :],
                            lhsT=msg[:, (lo + 2 * i) * D : (lo + 2 * i + w) * D],
                            rhs=ident[:],
                            start=(i == 0),
                            stop=(i == npair - 1),
                        )
                    if dj >= 2:
                        nc.vector.tensor_tensor(
                            out=aggT[:], in0=agg_ps[:D, :], in1=agg_ps[D:, :],
                            op=mybir.AluOpType.add)
                    else:
                        nc.vector.tensor_copy(out=aggT[:], in_=agg_ps[:D, :])
                else:
                    agg_ps = psA.tile([D, P], F32, tag="agg")
                    for dd in range(dj):
                        nc.tensor.matmul(
                            out=agg_ps[:],
                            lhsT=msg[:, (lo + dd) * D : (lo + dd + 1) * D],
                            rhs=ident[:],
                            start=(dd == 0),
                            stop=(dd == dj - 1),
                        )
                    if aggT_on_dve:
                        nc.vector.tensor_copy(out=aggT[:], in_=agg_ps[:])
                    else:
                        nc.scalar.copy(out=aggT[:], in_=agg_ps[:])
                d_ps = psB.tile([P, D], F32, tag="mm")
                nc.tensor.matmul(out=d_ps[:], lhsT=aggT[:], rhs=wt_sb[:],
                                 start=True, stop=True)
                nc.scalar.copy(out=dgrp[:, (j - blo) * D : (j - blo + 1) * D],
                               in_=d_ps[:])
                lo += dj

            nc.sync.dma_start(out=out_t.ap()[:, blo * D : bhi * D], in_=dgrp[:])
    nc.compile()
    return nc


def run_gcn(x, W, edge_weights, source, target, num_nodes, trace=False, bufs=9):
    """Full-input host entry: preprocess, build, run on 8 cores, assemble output."""
    n_nodes = int(num_nodes)
    x_f32 = np.ascontiguousarray(np.asarray(x, dtype=np.float32))
    # int8 row quantization; the per-row dequant scale folds into edge weights
    row_absmax = np.abs(x_f32).max(axis=1)
    scale = np.where(row_absmax > 0, row_absmax / 127.0, 1.0).astype(np.float32)
    x_i8 = np.clip(np.rint(x_f32 / scale[:, None]), -127, 127).astype(np.int8)

    pp = preprocess(source, target, edge_weights, n_nodes, src_scale=scale)
    nc = build_nc(pp, n_nodes, bufs=bufs)
    wt_np = np.ascontiguousarray(np.asarray(W, dtype=np.float32).T).astype(NP_BF16)
    eye_np = np.eye(P, dtype=np.float32).astype(NP_BF16)
    in_maps = []
    for k in range(N_CORES):
        pc = pp["per_core"][k]
        im = {"wT": wt_np, "eye": eye_np, "idx": pc["idx16"], "wgt": pc["w_all"]}
        for gi, uniq in enumerate(pc["uniq_list"]):
            tab = np.zeros((TABLE_ROWS, 4 * D), dtype=np.int8)
            tab[: len(uniq), :D] = x_i8[uniq]
            im[f"xg{gi}"] = tab
        in_maps.append(im)
    res = run_bass_kernel_spmd(nc, in_maps, core_ids=list(range(N_CORES)), trace=trace)

    nt, nblk = pp["nt"], pp["nblk"]
    out = np.empty((n_nodes, D), dtype=np.float32)
    for k in range(N_CORES):
        raw = np.asarray(res.results[k]["out"], dtype=np.float32)  # [128, nblk*64]
        cube = raw.reshape(P, nblk, D).transpose(1, 0, 2).reshape(nblk * P, D)
        perm_pad = pp["per_core"][k]["perm_pad"]
        valid = perm_pad >= 0
        out[k * nt + perm_pad[valid]] = cube[valid]
    return out, res


def kernel(**inputs) -> np.ndarray:
    """Harness entry: full unsharded inputs -> full (num_nodes, 64) output."""
    out, _ = run_gcn(
        np.asarray(inputs["x"]),
        np.asarray(inputs["W"]),
        np.asarray(inputs["edge_weights"]),
        np.asarray(inputs["source"]),
        np.asarray(inputs["target"]),
        int(inputs["num_nodes"]),
        trace=False,
    )
    return out


# revision 40
# speedup vs baseline: 29.7996x; 1.0772x over previous
"""GCN message-passing Bass kernel for TRN2 (8 cores).

Math: delta = segment_sum(w_e * x[src_e]) @ W^T   (linearity: transform after aggregate)

Sharding: targets split across 8 cores (12500 each). Per core, targets are
degree-sorted and grouped into 128-target blocks; block j gives each of its
128 targets D_j padded edge slots (pad -> weight 0). Blocks are packed into
gathers of <= 64 slots (8192 indices).

Per gather: ONE InstDMAGatherAnt pulls bf16 x rows from a per-gather DRAM
table (the unique x rows of that gather's slot entries, host-reindexed to
int16 ids, rows padded to 256B stride with a 128B payload):
  gt[p, s*64:(s+1)*64] = table[idx[s*128+p], :64]
DVE multiplies the gather in-place by per-slot weights (broadcast AP).
PE does the slot reduction: per block, dj accumulating matmuls
(lhsT=slot tile, rhs=identity) sum slot^T into PSUM [64,128]; Act copies the
f32 PSUM to bf16 SBUF; PE applies W^T (aggT as lhsT); Act copies the result
into a per-gather output tile written to DRAM contiguously in block order
(plain DMA, no scatter). The host applies the inverse target permutation to
assemble the final (num_nodes, 64) output.
"""

import math
from contextlib import ExitStack

import numpy as np
import ml_dtypes

import concourse.bass as bass
import concourse.bacc as bacc
import concourse.mybir as mybir
import concourse.tile as tile
from concourse.bass_utils import run_bass_kernel_spmd
from concourse.library_config import mlp as mlp_library

P = 128
N_CORES = 8
D = 64
F32 = mybir.dt.float32
BF16 = mybir.dt.bfloat16
I16 = mybir.dt.int16
I8 = mybir.dt.int8
NP_BF16 = np.dtype(ml_dtypes.bfloat16)

GATHER_SLOTS = 64  # max slots per dma_gather (64*128 = 8192 indices)
TABLE_ROWS = 8192  # per-gather unique-row table size (int16-indexable)


def preprocess(source, target, edge_weights, n_nodes, n_cores=N_CORES,
               src_scale=None):
    """Build per-core gather tables/indices/weights and the shared schedule.

    Returns dict with:
      d_sched: [nblk] per-block slot count (same for all cores)
      gathers: list of (blo, bhi, s0, gsz) gather groups over blocks
      per_core: dict with tables [G,TABLE_ROWS,128] bf16, idx16 [128,S*8] i16,
                w_all [128,S] bf16, perm_pad [nblk*128] i64
      nt, nblk, S
    """
    source = np.asarray(source).astype(np.int64)
    target = np.asarray(target).astype(np.int64)
    edge_weights = np.asarray(edge_weights).astype(np.float32)
    nt = n_nodes // n_cores
    assert nt * n_cores == n_nodes
    nblk = math.ceil(nt / P)
    ntp = nblk * P

    cores = []
    d_sched = np.zeros(nblk, dtype=np.int64)
    for k in range(n_cores):
        lo, hi = k * nt, (k + 1) * nt
        m = (target >= lo) & (target < hi)
        src_k = source[m]
        w_k = edge_weights[m]
        tl_k = target[m] - lo  # local target ids

        deg = np.bincount(tl_k, minlength=nt)
        perm = np.argsort(deg, kind="stable")  # local ids, degree-ascending
        deg_pad = np.concatenate([deg[perm], np.zeros(ntp - nt, dtype=deg.dtype)])
        d_k = deg_pad.reshape(nblk, P).max(axis=1)
        d_sched = np.maximum(d_sched, d_k)
        cores.append(dict(src=src_k, w=w_k, tl=tl_k, deg=deg, perm=perm))

    offs = np.concatenate([[0], np.cumsum(d_sched)]).astype(np.int64)
    S = int(offs[-1])

    # pack blocks into gathers of <= GATHER_SLOTS slots; keep the first and
    # last few gathers single-block so the pipeline head fill and the
    # un-overlapped tail stay short
    single_tail = 4
    gathers = []
    blo = 0
    while blo < nblk:
        bhi = blo
        gsz = 0
        while (bhi < nblk and (bhi == blo or gsz + d_sched[bhi] <= GATHER_SLOTS)
               and not (bhi > blo and bhi >= nblk - single_tail)):
            gsz += int(d_sched[bhi])
            bhi += 1
        gathers.append((blo, bhi, int(offs[blo]), gsz))
        blo = bhi
    G = len(gathers)

    per_core = []
    for k in range(n_cores):
        c = cores[k]
        deg, perm = c["deg"], c["perm"]
        rank = np.empty(nt, dtype=np.int64)
        rank[perm] = np.arange(nt)

        order = np.argsort(c["tl"], kind="stable")
        tls = c["tl"][order]
        srcs = c["src"][order]
        ws = c["w"][order]
        starts = np.cumsum(deg) - deg  # first edge position per target
        eo = np.arange(len(tls)) - starts[tls]  # occurrence index within target
        rr = rank[tls]
        pp = rr & (P - 1)
        bb = rr >> 7
        col = offs[bb] + eo

        entries = np.zeros((P, S), dtype=np.int64)  # pad -> x row 0 (weight 0)
        w_all = np.zeros((P, S), dtype=NP_BF16)
        entries[pp, col] = srcs
        # fold the int8 per-source-row dequant scale into the edge weight
        wsf = ws if src_scale is None else ws * src_scale[srcs].astype(np.float32)
        w_all[pp, col] = wsf.astype(NP_BF16)

        # per-gather unique tables + int16 indices, wrapped for the Q7 layout
        uniq_list = []
        idx16 = np.empty((P, S * 8), dtype=np.int16)
        for gi, (_, _, s0, gsz) in enumerate(gathers):
            ent = entries[:, s0 : s0 + gsz]
            uniq, inv = np.unique(ent, return_inverse=True)
            assert len(uniq) <= TABLE_ROWS
            uniq_list.append(uniq)
            inv = inv.reshape(P, gsz).astype(np.int16)
            iflat = inv.T.reshape(-1)  # position i = s_local*128 + p
            blkcols = np.tile(iflat.reshape(gsz * 8, 16).T, (8, 1))
            idx16[:, s0 * 8 : (s0 + gsz) * 8] = blkcols

        perm_pad = np.full(ntp, -1, dtype=np.int64)
        perm_pad[:nt] = perm
        per_core.append(dict(uniq_list=uniq_list, idx16=idx16, w_all=w_all,
                             perm_pad=perm_pad))

    return dict(d_sched=[int(d) for d in d_sched], S=S, gathers=gathers,
                per_core=per_core, nt=nt, nblk=nblk, G=G)


def _dma_gather(gp, out_ap, in_ap, idxs_ap, num_idxs):
    """InstDMAGatherAnt with a 128B payload at 256B row stride (elem_size=64
    bf16, stride_bytes_256=1). bass.dma_gather asserts elem%256B, but the Q7
    ucode handles 128B payloads (verified on HW); construct directly."""
    _in_ap = gp.lower_ap_dma(in_ap, for_custom_bir_dma=True)
    _idxs_ap = gp.lower_ap(idxs_ap)
    _out_ap = gp.lower_ap(out_ap)
    return gp.add_instruction(
        mybir.InstDMAGatherAnt(
            name=gp.bass.get_next_instruction_name(),
            ins=[*_in_ap, _idxs_ap, gp.lower_val_access(gp.to_reg(num_idxs))],
            outs=[_out_ap],
            transpose=False,
            num_idxs=num_idxs,
            elem_size=D,
            stride_bytes_256=1,
            gen_mode=0,
            single_packet=False,
            queue_num=0,
            sbuf_tokens_per_rank=0,
            sbuf_free_dim_per_rank=0,
            sbuf_free_dim_pad_per_rank=0,
            sbuf_byte_offset=0,
        )
    )


def build_nc(pp, n_nodes, bufs=9, out_bf16=True, psum_bufs=4, stages=3,
             aggT_on_dve=False, pair_transpose=False):
    # stages: 1=gather only, 2=+mult, 3=full (ablation knob for timing)
    d_sched, S, nblk, gathers = pp["d_sched"], pp["S"], pp["nblk"], pp["gathers"]
    nc = bacc.Bacc("TRN2", target_bir_lowering=False, debug=False)
    # int8 rows, padded to a 256B stride; payload = first 64 bytes
    tabs = [nc.dram_tensor(f"xg{gi}", [TABLE_ROWS, 4 * D], I8, kind="ExternalInput")
            for gi in range(len(gathers))]
    wt_t = nc.dram_tensor("wT", [D, D], BF16, kind="ExternalInput")
    idx_t = nc.dram_tensor("idx", [P, S * 8], I16, kind="ExternalInput")
    wgt_t = nc.dram_tensor("wgt", [P, S], BF16, kind="ExternalInput")
    eye_t = nc.dram_tensor("eye", [P, P], BF16, kind="ExternalInput")
    out_dt = BF16 if out_bf16 else F32
    # transposed output layout: [64 features, nblk*128 block-order targets]
    out_t = nc.dram_tensor("out", [D, nblk * P], out_dt, kind="ExternalOutput")

    with tile.TileContext(nc) as tc, ExitStack() as ctx:
        nc.gpsimd.load_library(mlp_library)
        const = ctx.enter_context(tc.tile_pool(name="const", bufs=1))
        gpool = ctx.enter_context(tc.tile_pool(name="gather", bufs=bufs))
        mpool = ctx.enter_context(tc.tile_pool(name="msg", bufs=bufs))
        tpool = ctx.enter_context(tc.tile_pool(name="aggT", bufs=8))
        dpool = ctx.enter_context(tc.tile_pool(name="delta", bufs=bufs))
        psa_bufs, psb_bufs = (psum_bufs if isinstance(psum_bufs, (tuple, list))
                              else (psum_bufs, psum_bufs))
        psA = ctx.enter_context(tc.tile_pool(name="psA", bufs=psa_bufs, space="PSUM"))
        psB = ctx.enter_context(tc.tile_pool(name="psB", bufs=psb_bufs, space="PSUM"))

        ident = const.tile([P, P], BF16)
        nc.sync.dma_start(out=ident[:], in_=eye_t.ap())
        wt_sb = const.tile([D, D], BF16)
        nc.sync.dma_start(out=wt_sb[:], in_=wt_t.ap())
        # two-slice idx load: the first gather only waits for its own slice
        idx_sb = const.tile([P, S * 8], I16)
        g0 = gathers[0][3] * 8
        nc.sync.dma_start(out=idx_sb[:, :g0], in_=idx_t.ap()[:, :g0])
        nc.sync.dma_start(out=idx_sb[:, g0:], in_=idx_t.ap()[:, g0:])
        wgt_sb = const.tile([P, S], BF16)
        nc.sync.dma_start(out=wgt_sb[:], in_=wgt_t.ap())

        # Prime engines on the upfront loads so per-block instructions carry
        # at most one sync wait each (SEQ instruction structs encode one).
        prime = const.tile([P, 1], BF16)
        nc.vector.tensor_copy(out=prime[:], in_=wgt_sb[:, :1])
        prime2 = const.tile([P, 1], BF16)
        nc.scalar.copy(out=prime2[:], in_=ident[:, :1])
        prime_ps = psA.tile([D, P], F32, tag="agg")
        nc.tensor.matmul(out=prime_ps[:], lhsT=ident[:, :D], rhs=ident[:],
                         start=True, stop=True)

        for gi, (blo, bhi, s0, gsz) in enumerate(gathers):
            gt = gpool.tile([P, gsz * D], I8, tag="g")
            _dma_gather(
                nc.gpsimd,
                gt[:].rearrange("p (c e) -> p c e", e=D),
                tabs[gi].ap(),
                idx_sb[:, s0 * 8 : (s0 + gsz) * 8],
                gsz * P,
            )
            if stages < 2:
                continue
            msg = mpool.tile([P, gsz * D], BF16, tag="m")
            nc.vector.tensor_tensor(
                out=msg[:].rearrange("p (d o) -> p d o", o=D),
                in0=gt[:].rearrange("p (d o) -> p d o", o=D),
                in1=wgt_sb[:, s0 : s0 + gsz].to_broadcast([P, gsz, D]),
                op=mybir.AluOpType.mult,
            )
            if stages < 3:
                continue

            ng = bhi - blo
            # output kept TRANSPOSED on device: dgrp [64 o, ng*128 t]; the
            # host transposes back. W is the stationary matmul operand so one
            # matmul transforms up to 4 blocks (rhs free dim <= 512).
            dgrp = dpool.tile([D, ng * P], out_dt, tag="d")
            lo = 0
            j = blo
            while j < bhi:
                nb = min(4, bhi - j)  # blocks in this W-matmul batch
                aggT = tpool.tile([D, nb * P], BF16, tag="aT")
                for b in range(nb):
                    dj = d_sched[j + b]
                    agg_ps = psA.tile([D, P], F32, tag="agg")
                    for dd in range(dj):
                        nc.tensor.matmul(
                            out=agg_ps[:],
                            lhsT=msg[:, (lo + dd) * D : (lo + dd + 1) * D],
                            rhs=ident[:],
                            start=(dd == 0),
                            stop=(dd == dj - 1),
                        )
                    nc.scalar.copy(out=aggT[:, b * P : (b + 1) * P], in_=agg_ps[:])
                    lo += dj
                dT_ps = psB.tile([D, nb * P], F32, tag="mm")
                nc.tensor.matmul(out=dT_ps[:], lhsT=wt_sb[:], rhs=aggT[:],
                                 start=True, stop=True)
                nc.scalar.copy(
                    out=dgrp[:, (j - blo) * P : (j - blo + nb) * P], in_=dT_ps[:])
                j += nb

            nc.sync.dma_start(out=out_t.ap()[:, blo * P : bhi * P], in_=dgrp[:])
    nc.compile()
    return nc


def run_gcn(x, W, edge_weights, source, target, num_nodes, trace=False, bufs=9):
    """Full-input host entry: preprocess, build, run on 8 cores, assemble output."""
    n_nodes = int(num_nodes)
    x_f32 = np.ascontiguousarray(np.asarray(x, dtype=np.float32))
    # int8 row quantization; the per-row dequant scale folds into edge weights
    row_absmax = np.abs(x_f32).max(axis=1)
    scale = np.where(row_absmax > 0, row_absmax / 127.0, 1.0).astype(np.float32)
    x_i8 = np.clip(np.rint(x_f32 / scale[:, None]), -127, 127).astype(np.int8)

    pp = preprocess(source, target, edge_weights, n_nodes, src_scale=scale)
    nc = build_nc(pp, n_nodes, bufs=bufs)
    wt_np = np.ascontiguousarray(np.asarray(W, dtype=np.float32).T).astype(NP_BF16)
    eye_np = np.eye(P, dtype=np.float32).astype(NP_BF16)
    in_maps = []
    for k in range(N_CORES):
        pc = pp["per_core"][k]
        im = {"wT": wt_np, "eye": eye_np, "idx": pc["idx16"], "wgt": pc["w_all"]}
        for gi, uniq in enumerate(pc["uniq_list"]):
            tab = np.zeros((TABLE_ROWS, 4 * D), dtype=np.int8)
            tab[: len(uniq), :D] = x_i8[uniq]
            im[f"xg{gi}"] = tab
        in_maps.append(im)
    res = run_bass_kernel_spmd(nc, in_maps, core_ids=list(range(N_CORES)), trace=trace)

    nt, nblk = pp["nt"], pp["nblk"]
    out = np.empty((n_nodes, D), dtype=np.float32)
    for k in range(N_CORES):
        raw = np.asarray(res.results[k]["out"], dtype=np.float32)  # [64, nblk*128]
        cube = raw.T  # [nblk*128, 64], row j*128+p = (block j, partition p)
        perm_pad = pp["per_core"][k]["perm_pad"]
        valid = perm_pad >= 0
        out[k * nt + perm_pad[valid]] = cube[valid]
    return out, res


def kernel(**inputs) -> np.ndarray:
    """Harness entry: full unsharded inputs -> full (num_nodes, 64) output."""
    out, _ = run_gcn(
        np.asarray(inputs["x"]),
        np.asarray(inputs["W"]),
        np.asarray(inputs["edge_weights"]),
        np.asarray(inputs["source"]),
        np.asarray(inputs["target"]),
        int(inputs["num_nodes"]),
        trace=False,
    )
    return out
